# revision 1
# baseline (speedup 1.0000x reference)
"""GroupedAttention Trainium2 kernel.

Problem: x[2048, 2, 256]; K/V projections to G=2 groups (head width 256),
Q projection to G*SUB=8 heads; LayerNorm on K and Q; causal softmax
attention per (b, g, sub); output projection back to 256.

Sharding: 16 (b, g, sub) heads over 8 cores -> 2 heads per core.
Core c: b = c//4, g = (c//2)%2, sub-pair j = c%2 (subs 2j, 2j+1).
Each core computes its K/V projections (one (b,g) group), its two Q
heads, attention, and a partial output projection. The host sums the 4
partials per batch (the WO contraction is a sum over head slices) and
adds a folded constant bias (WO_b + sum_h V_bias_h @ WO_slice_h).

On-device layout: scores are computed transposed, ST[k, q] =
(KT chunk).T @ QT, so the post-softmax P[k, q] feeds the PV matmul
directly as the stationary operand (no transpose in the inner loop).
A ones-column appended to V makes PSUM column 256 accumulate the
softmax denominator for free; normalization folds into a per-partition
multiply after the output projection. Causal masking multiplies P by
one of four precomputed 0/1 masks (on GpSimd, which is otherwise idle).
LayerNorm mean arrives free via a host-appended -mean column in each
weight matrix; variance comes from one Square-activation with accum_out.
All matmuls run in float32r (1 cycle/row at moving>=256, vs 4 for fp32).
"""

import sys

import numpy as np

for _p in ("/opt/trn_rl_repo",):
    if _p not in sys.path:
        sys.path.insert(0, _p)

SEQ, BS, DIM = 2048, 2, 256
G, SUB = 2, 4
N_CORES = 8
LN_EPS = 1e-5
NT = SEQ // 128  # 16 seq tiles of 128
NSB = SEQ // 512  # 4 query superblocks of 512

_CACHE = {}


def _build_program():
    from contextlib import ExitStack

    import concourse.bacc as bacc
    import concourse.mybir as mybir
    from concourse import tile
    f32 = mybir.dt.float32
    f32r = mybir.dt.float32r
    AF = mybir.ActivationFunctionType
    OP = mybir.AluOpType

    nc = bacc.Bacc("TRN2", target_bir_lowering=False, debug=False)

    xt_d = nc.dram_tensor("xt", [128, 2, SEQ], f32r, kind="ExternalInput").ap()
    wk_d = nc.dram_tensor("wk", [128, 2, 258], f32r, kind="ExternalInput").ap()
    bk_d = nc.dram_tensor("bk", [1, 258], f32r, kind="ExternalInput").ap()
    wv_d = nc.dram_tensor("wv", [128, 2, 256], f32r, kind="ExternalInput").ap()
    wq_d = nc.dram_tensor("wq", [128, 4, 258], f32r, kind="ExternalInput").ap()
    bq_d = nc.dram_tensor("bq", [1, 2, 258], f32r, kind="ExternalInput").ap()
    wo_d = nc.dram_tensor("wo", [128, 4, 256], f32r, kind="ExternalInput").ap()
    lng_d = nc.dram_tensor("lng", [128, 2], f32, kind="ExternalInput").ap()
    id_d = nc.dram_tensor("ident", [128, 128], f32r, kind="ExternalInput").ap()
    cm_d = nc.dram_tensor("cmask", [128, 4, 512], f32r, kind="ExternalInput").ap()
    on_d = nc.dram_tensor("onesrow", [1, SEQ], f32r, kind="ExternalInput").ap()
    vo_d = nc.dram_tensor("vpones", [128, NT, 2], f32r, kind="ExternalInput").ap()
    out_d = nc.dram_tensor("out_partial", [SEQ, DIM], f32, kind="ExternalOutput").ap()

    r = lambda ap: ap.bitcast(f32r)

    with tile.TileContext(nc) as tc, ExitStack() as ctx:
        const = ctx.enter_context(tc.tile_pool(name="const", bufs=1))

        xt_sb = const.tile([128, 2, SEQ], f32r)
        wk_sb = const.tile([128, 2, 258], f32r)
        bk_sb = const.tile([1, 258], f32r)
        wv_sb = const.tile([128, 2, 256], f32r)
        wq_sb = const.tile([128, 4, 258], f32r)
        bq_sb = const.tile([1, 2, 258], f32r)
        wo_sb = const.tile([128, 4, 256], f32r)
        lng_sb = const.tile([128, 2], f32)
        ident_sb = const.tile([128, 128], f32r)
        ones_sb = const.tile([1, SEQ], f32r)
        zero_sb = const.tile([128, 1], f32)
        eps_sb = const.tile([128, 1], f32)

        # per-tile tensors for exact dependency tracking
        kt_t = [
            [const.tile([128, 128], f32r, name=f"ktt{c}_{t}") for t in range(NT)]
            for c in range(2)
        ]
        qt_t = [
            [const.tile([128, 512], f32r, name=f"qtt{ci}_{s}") for s in range(NSB)]
            for ci in range(4)
        ]
        vp_t = [const.tile([128, 258], f32r, name=f"vpt{t}") for t in range(NT)]
        ot_t = [
            [
                [const.tile([128, 128], f32r, name=f"ott{c}_{s}_{j}") for j in range(4)]
                for s in range(NSB)
            ]
            for c in range(4)
        ]
        masks_t = [const.tile([128, 512], f32r, name=f"mask{o}") for o in range(4)]

        nc.sync.dma_start(xt_sb[:], xt_d[:])
        nc.sync.dma_start(wk_sb[:], wk_d[:])
        nc.sync.dma_start(bk_sb[:], bk_d[:])
        nc.sync.dma_start(wv_sb[:], wv_d[:])
        nc.sync.dma_start(wq_sb[:], wq_d[:])
        nc.sync.dma_start(bq_sb[:], bq_d[:])
        nc.sync.dma_start(wo_sb[:], wo_d[:])
        nc.sync.dma_start(lng_sb[:], lng_d[:])
        nc.sync.dma_start(ident_sb[:], id_d[:])
        for o in range(4):
            nc.sync.dma_start(masks_t[o][:], cm_d[:, o, :])
        nc.sync.dma_start(ones_sb[:], on_d[:])
        for t in range(NT):
            nc.sync.dma_start(vp_t[t][:, 256:258], vo_d[:, t, :])
        nc.gpsimd.memset(zero_sb[:], 0.0)
        nc.gpsimd.memset(eps_sb[:], LN_EPS)

        psA = ctx.enter_context(tc.tile_pool(name="psA", bufs=3, space="PSUM"))
        psB = ctx.enter_context(tc.tile_pool(name="psB", bufs=1, space="PSUM"))
        psT = ctx.enter_context(tc.tile_pool(name="psT", bufs=1, space="PSUM"))
        wrk = ctx.enter_context(tc.tile_pool(name="wrk", bufs=3))
        ppool = ctx.enter_context(tc.tile_pool(name="ppool", bufs=3))
        opool = ctx.enter_context(tc.tile_pool(name="opool", bufs=2))

        def v_proj(t):
            vps = psA.tile([128, 256], f32, tag="mm512", name=f"vps{t}")
            for c in range(2):
                nc.tensor.matmul(
                    vps[:],
                    lhsT=r(xt_sb[:, c, t * 128 : (t + 1) * 128]),
                    rhs=r(wv_sb[:, c, :]),
                    start=(c == 0),
                    stop=(c == 1),
                )
            nc.vector.tensor_copy(vp_t[t][:, 0:256], vps[:])

        def ln_proj(hi, t, wchunks, brow, dest_write):
            """project seq-tile t, layernorm, transpose; dest_write(c, tp_psum)"""
            pps = psA.tile([128, 258], f32, tag="mm512", name=f"pps{hi}_{t}")
            nc.tensor.matmul(
                pps[:],
                lhsT=r(xt_sb[:, 0, t * 128 : (t + 1) * 128]),
                rhs=r(wchunks[0]),
                start=True,
                stop=False,
            )
            nc.tensor.matmul(
                pps[:],
                lhsT=r(xt_sb[:, 1, t * 128 : (t + 1) * 128]),
                rhs=r(wchunks[1]),
                start=False,
                stop=False,
            )
            nc.tensor.matmul(
                pps[:],
                lhsT=r(ones_sb[0:1, t * 128 : (t + 1) * 128]),
                rhs=r(brow),
                start=False,
                stop=True,
            )
            # col 256 of pps = -mean (host appended -mean weight column)
            mneg = wrk.tile([128, 1], f32, tag="mneg", name=f"mneg{hi}_{t}")
            nc.vector.tensor_copy(mneg[:], pps[:, 256:257])
            sq = wrk.tile([128, 256], f32, tag="sq", name=f"sq{hi}_{t}")
            var = wrk.tile([128, 1], f32, tag="var", name=f"var{hi}_{t}")
            nc.scalar.activation(
                sq[:], pps[:, 0:256], AF.Square, bias=mneg[:], accum_out=var[:]
            )
            std = wrk.tile([128, 1], f32, tag="std", name=f"std{hi}_{t}")
            nc.scalar.activation(
                std[:], var[:], AF.Sqrt, bias=eps_sb[:], scale=1.0 / 256.0
            )
            rstd = wrk.tile([128, 1], f32, tag="rstd", name=f"rstd{hi}_{t}")
            nc.vector.reciprocal(rstd[:], std[:])
            kn = wrk.tile([128, 256], f32r, tag="kn", name=f"kn{hi}_{t}")
            nc.vector.tensor_scalar(
                kn[:],
                pps[:, 0:256],
                scalar1=mneg[:],
                scalar2=rstd[:],
                op0=OP.add,
                op1=OP.mult,
            )
            for c in range(2):
                tp = psT.tile([128, 128], f32r, tag="tp", name=f"tp{hi}_{t}_{c}")
                nc.tensor.transpose(
                    tp[:], r(kn[:, c * 128 : (c + 1) * 128]), r(ident_sb[:])
                )
                dest_write(c, tp)

        def k_write(t):
            def w(c, tp):
                nc.vector.tensor_scalar_mul(
                    kt_t[c][t][:], tp[:].bitcast(f32), lng_sb[:, c : c + 1]
                )

            return w

        def q_write(cbase, t):
            def w(c, tp):
                nc.vector.tensor_scalar_mul(
                    qt_t[cbase + c][t // 4][:, (t % 4) * 128 : (t % 4 + 1) * 128],
                    tp[:].bitcast(f32),
                    lng_sb[:, c : c + 1],
                )

            return w

        # round-robin projections so attention can start after 4 seq-tiles
        for t in range(NT):
            ln_proj(0, t, [wk_sb[:, 0, :], wk_sb[:, 1, :]], bk_sb[0:1, :], k_write(t))
            ln_proj(
                1, t, [wq_sb[:, 0, :], wq_sb[:, 1, :]], bq_sb[0:1, 0, :], q_write(0, t)
            )
            v_proj(t)
            ln_proj(
                2, t, [wq_sb[:, 2, :], wq_sb[:, 3, :]], bq_sb[0:1, 1, :], q_write(2, t)
            )

        # ---- attention: heads interleaved at superblock granularity ----
        def attn_superblock(h, s):
            n_k = 4 * (s + 1)
            oacc = [
                psB.tile([128, 258], f32, tag=f"oacc{j}", name=f"oacc{h}_{s}_{j}")
                for j in range(4)
            ]
            for kt in range(n_k):
                st = psA.tile([128, 512], f32, tag="mm512", name=f"st{h}_{s}_{kt}")
                for c in range(2):
                    nc.tensor.matmul(
                        st[:],
                        lhsT=r(kt_t[c][kt][:]),
                        rhs=r(qt_t[h * 2 + c][s][:]),
                        start=(c == 0),
                        stop=(c == 1),
                    )
                p = ppool.tile([128, 512], f32r, tag="p", name=f"p{h}_{s}_{kt}")
                nc.scalar.activation(
                    p[:], st[:], AF.Exp, bias=zero_sb[:], scale=1.0 / 16.0
                )
                if kt >= n_k - 4:
                    o = kt - (n_k - 4)
                    nc.vector.tensor_mul(p[:], p[:], masks_t[o][:])
                for j in range(4):
                    nc.tensor.matmul(
                        oacc[j][:],
                        lhsT=r(p[:, j * 128 : (j + 1) * 128]),
                        rhs=r(vp_t[kt][:]),
                        start=(kt == 0),
                        stop=(kt == n_k - 1),
                    )
            for j in range(4):
                rc = wrk.tile([128, 1], f32, tag="rc", name=f"rc{h}_{s}_{j}")
                nc.vector.reciprocal(rc[:], oacc[j][:, 256:257])
                osb = opool.tile([128, 256], f32r, tag="osb", name=f"osb{h}_{s}_{j}")
                nc.vector.tensor_scalar_mul(osb[:], oacc[j][:, 0:256], rc[:])
                for c in range(2):
                    otp = psT.tile(
                        [128, 128], f32r, tag="tp", name=f"otp{h}_{s}_{j}_{c}"
                    )
                    nc.tensor.transpose(
                        otp[:], r(osb[:, c * 128 : (c + 1) * 128]), r(ident_sb[:])
                    )
                    nc.vector.tensor_copy(ot_t[h * 2 + c][s][j][:], otp[:].bitcast(f32))

        def o_proj(t):
            s, j = t // 4, t % 4
            ops = psB.tile([128, 256], f32, tag=f"oacc{t % 4}", name=f"ops{t}")
            for c in range(4):
                nc.tensor.matmul(
                    ops[:],
                    lhsT=r(ot_t[c][s][j][:]),
                    rhs=r(wo_sb[:, c, :]),
                    start=(c == 0),
                    stop=(c == 3),
                )
            outsb = opool.tile([128, 256], f32, tag="outsb", name=f"outsb{t}")
            nc.scalar.copy(outsb[:], ops[:])
            nc.sync.dma_start(out_d[t * 128 : (t + 1) * 128, :], outsb[:])

        for s in range(NSB):
            for h in range(2):
                attn_superblock(h, s)
            for t in range(4 * s, 4 * s + 4):
                o_proj(t)

    nc.finalize()
    return nc


def _chunk2(a):
    """[256, F] -> [128, 2, F] (feature chunks on the free axis)."""
    f = a.shape[1]
    return np.ascontiguousarray(a.reshape(2, 128, f).transpose(1, 0, 2))


def _prep_core_inputs(c, x, WK_w, WK_b, WV_w, WV_b, WQ_w, WQ_b, WO_w, ln_g):
    b, g, j = c // 4, (c // 2) % 2, c % 2
    f32 = np.float32

    xT = np.ascontiguousarray(x[:, b, :].T.astype(f32))  # [256, 2048]
    xt = _chunk2(xT)

    def ln_weight(w, bias):  # w [256, 256], bias [256] -> w' [256,258], b' [1,258]
        wm = -w.mean(axis=1, keepdims=True)
        zc = np.zeros_like(wm)
        wp = np.concatenate([w, wm, zc], axis=1).astype(f32)
        bp = np.concatenate([bias, [-bias.mean()], [0.0]]).astype(f32)[None, :]
        return wp, bp

    wk_s = WK_w[:, g * 256 : (g + 1) * 256]
    wkp, bkp = ln_weight(wk_s, WK_b[g * 256 : (g + 1) * 256])

    wv_s = np.ascontiguousarray(WV_w[:, g * 256 : (g + 1) * 256].astype(f32))

    wq_chunks, bq_rows = [], []
    for sh in (2 * j, 2 * j + 1):
        col = (g * SUB + sh) * 256
        wqp, bqp = ln_weight(WQ_w[:, col : col + 256], WQ_b[col : col + 256])
        wq_chunks.append(_chunk2(wqp))
        bq_rows.append(bqp)
    wq = np.ascontiguousarray(
        np.concatenate(
            [wq_chunks[0], wq_chunks[1]], axis=1
        )  # [128, 2, 257] + [128, 2, 257] -> [128, 4, 257]
    )
    bq = np.ascontiguousarray(np.stack([bq_rows[0][0], bq_rows[1][0]])[None, :, :])

    row = (g * SUB + 2 * j) * 256
    wo_s = WO_w[row : row + 512, :].astype(f32)  # [512, 256]
    wo = np.ascontiguousarray(wo_s.reshape(4, 128, 256).transpose(1, 0, 2))

    lng = np.ascontiguousarray(ln_g.astype(f32).reshape(2, 128).T)

    f = np.float32
    pp, ff = np.meshgrid(np.arange(128), np.arange(512), indexing="ij")
    cmask = np.stack(
        [(o * 128 + pp <= ff).astype(f) for o in range(4)], axis=1
    )  # [128, 4, 512]
    return {
        "xt": xt,
        "wk": _chunk2(wkp),
        "bk": bkp,
        "wv": _chunk2(wv_s),
        "wq": wq,
        "bq": bq,
        "wo": wo,
        "lng": lng,
        "ident": np.eye(128, dtype=f),
        "cmask": np.ascontiguousarray(cmask),
        "onesrow": np.ones((1, SEQ), dtype=f),
        "vpones": np.concatenate(
            [np.ones((128, NT, 1), dtype=f), np.zeros((128, NT, 1), dtype=f)], axis=2
        ),
    }


def kernel(x, WK_w, WK_b, WV_w, WV_b, WQ_w, WQ_b, WO_w, WO_b, ln_g, ln_b, **kwargs):
    x = np.asarray(x)
    WK_w, WK_b = np.asarray(WK_w), np.asarray(WK_b)
    WV_w, WV_b = np.asarray(WV_w), np.asarray(WV_b)
    WQ_w, WQ_b = np.asarray(WQ_w), np.asarray(WQ_b)
    WO_w, WO_b = np.asarray(WO_w), np.asarray(WO_b)
    ln_g, ln_b = np.asarray(ln_g), np.asarray(ln_b)

    if not np.allclose(ln_b, 0.0):
        raise NotImplementedError("nonzero ln_b not supported by this kernel")

    if "nc" not in _CACHE:
        _CACHE["nc"] = _build_program()
    nc = _CACHE["nc"]

    in_maps = [
        _prep_core_inputs(c, x, WK_w, WK_b, WV_w, WV_b, WQ_w, WQ_b, WO_w, ln_g)
        for c in range(N_CORES)
    ]

    from concourse.bass_utils import run_bass_kernel_spmd

    res = run_bass_kernel_spmd(nc, in_maps, list(range(N_CORES)))
    results = res.results

    out = np.zeros((SEQ, BS, DIM), dtype=np.float32)
    for c in range(N_CORES):
        out[:, c // 4, :] += results[c]["out_partial"]

    # fold: WO_b plus the V-bias contribution of every head
    const_bias = WO_b.astype(np.float64).copy()
    for g in range(G):
        bv = WV_b[g * 256 : (g + 1) * 256].astype(np.float64)
        for sh in range(SUB):
            row = (g * SUB + sh) * 256
            const_bias += bv @ WO_w[row : row + 256, :].astype(np.float64)
    out += const_bias.astype(np.float32)[None, None, :]
    return out



# revision 23
# speedup vs baseline: 1.4548x; 1.4548x over previous
"""GroupedAttention Trainium2 kernel.

Problem: x[2048, 2, 256]; K/V projections to G=2 groups (head width 256),
Q projection to G*SUB=8 heads; LayerNorm on K and Q; causal softmax
attention per (b, g, sub); output projection back to 256.

Sharding: 16 (b, g, sub) heads over 8 cores -> 2 heads per core.
Core c: b = c//4, g = (c//2)%2, sub-pair j = c%2 (subs 2j, 2j+1).
The host sums the 4 partials per batch and adds a folded constant bias.

Key structure (all timings against the TRN2 cost model):
- K^T and Q^T are produced DIRECTLY by matmuls (weights stationary,
  x^T moving), eliminating every K/Q transpose on the PE.
- LayerNorm is never applied to K. Scores use raw (biased) K^T; the
  per-key factor 1/(16*std_k) folds into the Exp activation's
  per-partition scale AP, and the mean term cancels because the
  normalized Q rows sum to ~0 (ln_g == 1). Per-key mean/sumsq come from
  tiny N=1 matmuls against precomputed row-mean weight columns and a
  Square+ones-reduction, batched 4 seq-tiles per PSUM bank.
- Q IS normalized (its per-query scale sits inside the softmax):
  mean/sumsq rows are computed by M=2/M=1 matmuls into one [4,512]
  PSUM tile, converted to (mu, 1/std) rows, broadcast across partitions
  on the otherwise-idle GPSIMD engine, and applied with one
  scalar_tensor_tensor (bias add + mean subtract) plus one multiply.
- Causal structure at 128-tile granularity: score columns below the
  diagonal tile are skipped entirely (bf16 moving keeps 1 cyc/row even
  below 256 columns), PV matmuls for empty tile pairs are skipped, and
  only the diagonal 128x128 tile is masked (one shared 0/1 bf16 mask).
- A ones-column appended to V makes PSUM column 256 accumulate the
  softmax denominator for free.
- The kt loop is software-pipelined (scores two blocks ahead of PV);
  O-transposes are batched 4-per-bank with one wide PSUM->SBUF copy and
  injected, with the output projection, into the next superblock's
  instruction stream so the PE never waits on DVE chains.
- bf16 for Q^T-normalized, P, O tiles and the transpose identity
  (transposes run 1.0 cyc/row); everything accumulates in fp32 PSUM.
"""

import sys

import numpy as np

for _p in ("/opt/trn_rl_repo",):
    if _p not in sys.path:
        sys.path.insert(0, _p)

SEQ, BS, DIM = 2048, 2, 256
G, SUB = 2, 4
N_CORES = 8
LN_EPS = 1e-5
NT = SEQ // 128  # 16 seq tiles of 128
NSB = SEQ // 512  # 4 blocks of 512 (query superblocks / proj blocks)

_CACHE = {}


def _build_program():
    from contextlib import ExitStack

    import concourse.bacc as bacc
    import concourse.mybir as mybir
    from concourse import tile
    f32 = mybir.dt.float32
    f32r = mybir.dt.float32r
    bf16 = mybir.dt.bfloat16
    AF = mybir.ActivationFunctionType
    OP = mybir.AluOpType

    nc = bacc.Bacc("TRN2", target_bir_lowering=False, debug=False)

    xt_d = nc.dram_tensor("xt", [128, 2, SEQ], f32r, kind="ExternalInput").ap()
    hdr_d = nc.dram_tensor("hdr", [128, 2560], f32r, kind="ExternalInput").ap()
    bkc_d = nc.dram_tensor("bkc", [128, 2], f32, kind="ExternalInput").ap()
    wmk_d = nc.dram_tensor("wmk", [128, 4], f32r, kind="ExternalInput").ap()
    bkm_d = nc.dram_tensor("bkm", [128, 1], f32, kind="ExternalInput").ap()
    bqc_d = nc.dram_tensor("bqc", [128, 4], f32, kind="ExternalInput").ap()
    wmq_d = nc.dram_tensor("wmq", [128, 2, 2], f32r, kind="ExternalInput").ap()
    bqm_d = nc.dram_tensor("bqm", [1, 2], f32, kind="ExternalInput").ap()
    wv_d = nc.dram_tensor("wv", [128, 2, 256], f32r, kind="ExternalInput").ap()
    wo_d = nc.dram_tensor("wo", [128, 4, 256], bf16, kind="ExternalInput").ap()
    id_d = nc.dram_tensor("ident", [128, 128], bf16, kind="ExternalInput").ap()
    cm_d = nc.dram_tensor("cmask", [128, 128], bf16, kind="ExternalInput").ap()
    vo_d = nc.dram_tensor("vpones", [128, NT, 2], bf16, kind="ExternalInput").ap()
    out_d = nc.dram_tensor("out_partial", [SEQ, DIM], f32, kind="ExternalOutput").ap()

    r = lambda ap: ap.bitcast(f32r)

    with tile.TileContext(nc) as tc, ExitStack() as ctx:
        const = ctx.enter_context(tc.tile_pool(name="const", bufs=1))

        xt_sb = const.tile([128, 2, SEQ], f32r)
        hdr_sb = const.tile([128, 2560], f32r)
        bkc_sb = const.tile([128, 2], f32)
        wmk_sb = const.tile([128, 4], f32r)
        bkm_sb = const.tile([128, 1], f32)
        bqc_sb = const.tile([128, 4], f32)
        wmq_sb = const.tile([128, 2, 2], f32r)
        bqm_sb = const.tile([1, 2], f32)
        wv_sb = const.tile([128, 2, 256], f32r)
        wo_sb = const.tile([128, 4, 256], bf16)
        ident_sb = const.tile([128, 128], bf16)
        tmask_sb = const.tile([128, 128], bf16)
        onescol_sb = const.tile([128, 2], f32)
        epsk_sb = const.tile([128, 1], f32)
        epsq_sb = const.tile([1, 1], f32)

        # persistent SBUF activations
        ktb = [
            [const.tile([128, 512], bf16, name=f"ktb{f}_{b}") for b in range(NSB)]
            for f in range(2)
        ]
        qtn = [
            [const.tile([128, 512], bf16, name=f"qtn{fc}_{b}") for b in range(NSB)]
            for fc in range(4)
        ]
        vp_t = [const.tile([128, 258], bf16, name=f"vpt{t}") for t in range(NT)]
        rk16b = [const.tile([128, 8], f32, name=f"rk16b{b}") for b in range(NSB)]
        otb = [
            [const.tile([128, 512], bf16, name=f"otb{c}_{s}") for s in range(NSB)]
            for c in range(4)
        ]

        nc.gpsimd.memset(onescol_sb[:], 1.0)
        nc.gpsimd.memset(epsk_sb[:], 256.0 * LN_EPS)
        nc.gpsimd.memset(epsq_sb[:], LN_EPS)
        nc.sync.dma_start(hdr_sb[:, 0:1024], hdr_d[:, 0:1024])
        nc.sync.dma_start(hdr_sb[:, 1024:2560], hdr_d[:, 1024:2560])
        nc.sync.dma_start(bkc_sb[:], bkc_d[:])
        nc.sync.dma_start(wmk_sb[:], wmk_d[:])
        nc.sync.dma_start(bkm_sb[:], bkm_d[:])
        nc.sync.dma_start(bqc_sb[:], bqc_d[:])
        nc.sync.dma_start(wmq_sb[:], wmq_d[:])
        nc.sync.dma_start(bqm_sb[:], bqm_d[:])
        nc.sync.dma_start(wv_sb[:], wv_d[:])
        nc.sync.dma_start(xt_sb[:, :, 512:1024], xt_d[:, :, 512:1024])
        nc.sync.dma_start(xt_sb[:, :, 1024:1536], xt_d[:, :, 1024:1536])
        nc.sync.dma_start(xt_sb[:, :, 1536:2048], xt_d[:, :, 1536:2048])
        nc.sync.dma_start(ident_sb[:], id_d[:])
        nc.sync.dma_start(tmask_sb[:], cm_d[:])
        for t in range(NT):
            nc.sync.dma_start(vp_t[t][:, 256:258], vo_d[:, t, :])
        nc.sync.dma_start(wo_sb[:], wo_d[:])


        def wk_v(c, f):
            return hdr_sb[:, c * 256 + f * 128 : c * 256 + (f + 1) * 128]

        def wq_v(c, lo, hi):
            return hdr_sb[:, 1024 + c * 512 + lo : 1024 + c * 512 + hi]

        def xt_v(c, lo, hi):
            if hi <= 512:
                base = 512 if c == 0 else 2048
                return hdr_sb[:, base + lo : base + hi]
            return xt_sb[:, c, lo:hi]

        wrk = ctx.enter_context(tc.tile_pool(name="wrk", bufs=3))
        ppool = ctx.enter_context(tc.tile_pool(name="ppool", bufs=4))
        opool = ctx.enter_context(tc.tile_pool(name="opool", bufs=4))

        # ======== projection phase (pools scoped; close before attention) ====
        with tc.tile_pool(name="projp", bufs=3, space="PSUM") as projp, \
             tc.tile_pool(name="tinyp", bufs=1, space="PSUM") as tinyp, \
             tc.tile_pool(name="rowp", bufs=1, space="PSUM") as rowp:
            for B in range(NSB):
                sl = slice(B * 512, (B + 1) * 512)
                # --- K^T chunks: biased SBUF copy + biased square (from PSUM)
                ktsq = []
                for f in range(2):
                    kps = projp.tile([128, 512], f32, tag="proj", name=f"kps{f}_{B}")
                    for c in range(2):
                        nc.tensor.matmul(
                            kps[:],
                            lhsT=r(wk_v(c, f)),
                            rhs=r(xt_v(c, B * 512, (B + 1) * 512)),
                            start=(c == 0),
                            stop=(c == 1),
                        )
                    ksq = wrk.tile([128, 512], f32r, tag=f"ksq{f}", bufs=2,
                                   name=f"ksq{f}_{B}")
                    nc.scalar.activation(
                        ksq[:], kps[:], AF.Square, bias=bkc_sb[:, f : f + 1]
                    )
                    nc.vector.tensor_scalar_add(
                        ktb[f][B][:], kps[:], scalar1=bkc_sb[:, f : f + 1]
                    )
                    ktsq.append(ksq)
                # --- Q^T chunks: raw in PSUM until normalize; biased square
                qps_l, qtsq = [], []
                for fc in range(4):
                    qps = projp.tile([128, 512], f32, tag="proj", name=f"qps{fc}_{B}")
                    for c in range(2):
                        nc.tensor.matmul(
                            qps[:],
                            lhsT=r(wq_v(c, fc * 128, (fc + 1) * 128)),
                            rhs=r(xt_v(c, B * 512, (B + 1) * 512)),
                            start=(c == 0),
                            stop=(c == 1),
                        )
                    qsq = wrk.tile([128, 512], f32r, tag=f"qsq{fc}", bufs=2,
                                   name=f"qsq{fc}_{B}")
                    nc.scalar.activation(
                        qsq[:], qps[:], AF.Square, bias=bqc_sb[:, fc : fc + 1]
                    )
                    qps_l.append(qps)
                    qtsq.append(qsq)
                # --- V tiles ---
                for t in range(4 * B, 4 * B + 4):
                    vps = projp.tile([128, 256], f32, tag="proj", name=f"vps{t}")
                    for c in range(2):
                        nc.tensor.matmul(
                            vps[:],
                            lhsT=r(xt_v(c, t * 128, (t + 1) * 128)),
                            rhs=r(wv_sb[:, c, :]),
                            start=(c == 0),
                            stop=(c == 1),
                        )
                    nc.scalar.copy(vp_t[t][:, 0:256], vps[:])
                # --- K per-key stats: mu (cols 0:4) and sumsq (cols 4:8) ---
                tiny = tinyp.tile([128, 16], f32, tag="tiny", name=f"tiny{B}")
                for i, t in enumerate(range(4 * B, 4 * B + 4)):
                    for c in range(2):
                        nc.tensor.matmul(
                            tiny[:, 2 * i : 2 * i + 2],
                            lhsT=r(xt_v(c, t * 128, (t + 1) * 128)),
                            rhs=wmk_sb[:, 2 * c : 2 * c + 2],
                            start=(c == 0),
                            stop=(c == 1),
                        )
                    for f in range(2):
                        nc.tensor.matmul(
                            tiny[:, 8 + 2 * i : 10 + 2 * i],
                            lhsT=r(ktsq[f][:, i * 128 : (i + 1) * 128]),
                            rhs=r(onescol_sb[:, 0:2]),
                            start=(f == 0),
                            stop=(f == 1),
                        )
                # --- Q row stats, one bank per head: mu at partition 0,
                # sumsq at partition 32 (matmul base must be 0/32/64)
                mur_h = [
                    rowp.tile([1, 512], f32, tag=f"mur{h}", name=f"mur{h}_{B}")
                    for h in range(2)
                ]
                sqr_h = [
                    rowp.tile([1, 512], f32, tag=f"sqr{h}", name=f"sqr{h}_{B}")
                    for h in range(2)
                ]
                for h in range(2):
                    for c in range(2):
                        nc.tensor.matmul(
                            mur_h[h][:],
                            lhsT=wmq_sb[:, c, h : h + 1],
                            rhs=r(xt_v(c, B * 512, (B + 1) * 512)),
                            start=(c == 0),
                            stop=(c == 1),
                        )
                        nc.tensor.matmul(
                            sqr_h[h][:],
                            lhsT=r(onescol_sb[:, 0:1]),
                            rhs=r(qtsq[2 * h + c][:]),
                            start=(c == 0),
                            stop=(c == 1),
                        )
                # --- K stats -> rk16 (batched over the 4 seq tiles) ---
                mu2 = wrk.tile([128, 8], f32, tag="mu2", name=f"mu2_{B}")
                nc.scalar.activation(mu2[:], tiny[:, 0:8], AF.Square, bias=bkm_sb[:])
                v256 = wrk.tile([128, 8], f32, tag="v256", name=f"v256_{B}")
                nc.vector.scalar_tensor_tensor(
                    v256[:], mu2[:], -256.0, tiny[:, 8:16],
                    op0=OP.mult, op1=OP.add,
                )
                std16 = wrk.tile([128, 8], f32, tag="std16", name=f"std16_{B}")
                nc.scalar.activation(std16[:], v256[:], AF.Sqrt, bias=epsk_sb[:])
                nc.vector.reciprocal(rk16b[B][:], std16[:])
                # --- Q row stats -> (mu_biased, 1/std) rows + broadcasts ---
                mub = [None, None]
                rqb = [None, None]
                for h in range(2):
                    murow = wrk.tile([1, 512], f32, tag=f"murow{h}", bufs=1,
                                     name=f"murow{h}_{B}")
                    nc.vector.tensor_scalar_add(
                        murow[:], mur_h[h][:],
                        scalar1=bqm_sb[0:1, h : h + 1],
                    )
                    mu2r = wrk.tile([1, 512], f32, tag=f"mu2r{h}", bufs=1,
                                    name=f"mu2r{h}_{B}")
                    nc.scalar.activation(mu2r[:], murow[:], AF.Square)
                    v256r = wrk.tile([1, 512], f32, tag=f"v256r{h}", bufs=1,
                                     name=f"v256r{h}_{B}")
                    nc.vector.scalar_tensor_tensor(
                        v256r[:], mu2r[:], -256.0, sqr_h[h][:],
                        op0=OP.mult, op1=OP.add,
                    )
                    stdr = wrk.tile([1, 512], f32, tag=f"stdr{h}", bufs=1,
                                    name=f"stdr{h}_{B}")
                    nc.scalar.activation(
                        stdr[:], v256r[:], AF.Sqrt, bias=epsq_sb[:],
                        scale=1.0 / 256.0,
                    )
                    rqrow = wrk.tile([1, 512], f32, tag=f"rqrow{h}", bufs=1,
                                     name=f"rqrow{h}_{B}")
                    nc.vector.reciprocal(rqrow[:], stdr[:])
                    mub[h] = wrk.tile([128, 512], f32, tag=f"mub{h}", bufs=1,
                                      name=f"mub{h}_{B}")
                    nc.gpsimd.partition_broadcast(mub[h][:], murow[:])
                    rqb[h] = wrk.tile([128, 512], f32, tag=f"rqb{h}", bufs=1,
                                      name=f"rqb{h}_{B}")
                    nc.gpsimd.partition_broadcast(rqb[h][:], rqrow[:])
                # --- normalize Q: ((raw + bias) - mu) * (1/std) -> bf16 ---
                for fc in range(4):
                    h = fc // 2
                    qtmp = wrk.tile([128, 512], f32, tag=f"qtmp{fc % 2}",
                                    name=f"qtmp{fc}_{B}")
                    nc.vector.scalar_tensor_tensor(
                        qtmp[:], qps_l[fc][:], bqc_sb[:, fc : fc + 1], mub[h][:],
                        op0=OP.add, op1=OP.subtract,
                    )
                    eng = nc.vector if fc % 2 == 0 else nc.gpsimd
                    eng.tensor_mul(qtn[fc][B][:], qtmp[:], rqb[h][:])

        # ======== attention phase ========
        psA = ctx.enter_context(tc.tile_pool(name="psA", bufs=2, space="PSUM"))
        psB = ctx.enter_context(tc.tile_pool(name="psB", bufs=1, space="PSUM"))
        psT = ctx.enter_context(tc.tile_pool(name="psT", bufs=2, space="PSUM"))

        def attn_superblock(h, s, inject=None):
            n_k = 4 * (s + 1)
            oacc = [
                psB.tile([128, 258], f32, tag=f"oacc{j}", name=f"oacc{h}_{s}_{j}")
                for j in range(4)
            ]
            p_tiles = [None] * n_k

            def issue_scores(kt):
                d = kt - 4 * s  # >= 0 on the diagonal region
                qoff = 0 if d <= 0 else d * 128  # bf16 moving: 1 cyc/row anyway
                st = psA.tile([128, 512], f32, tag="mm512", name=f"st{h}_{s}_{kt}")
                for c in range(2):
                    nc.tensor.matmul(
                        st[:, qoff:512],
                        lhsT=ktb[c][kt // 4][:, (kt % 4) * 128 : (kt % 4 + 1) * 128],
                        rhs=qtn[h * 2 + c][s][:, qoff:512],
                        start=(c == 0),
                        stop=(c == 1),
                    )
                p = ppool.tile([128, 512], bf16, tag="p", name=f"p{h}_{s}_{kt}")
                nc.scalar.activation(
                    p[:, qoff:512], st[:, qoff:512], AF.Exp,
                    scale=rk16b[kt // 4][:, 2 * (kt % 4) : 2 * (kt % 4) + 1],
                )
                if d >= 0:
                    nc.vector.tensor_mul(
                        p[:, d * 128 : (d + 1) * 128],
                        p[:, d * 128 : (d + 1) * 128],
                        tmask_sb[:],
                    )
                p_tiles[kt] = p

            def issue_pv(kt):
                d = kt - 4 * s
                p = p_tiles[kt]
                for j in range(max(d, 0), 4):
                    nc.tensor.matmul(
                        oacc[j][:],
                        lhsT=p[:, j * 128 : (j + 1) * 128],
                        rhs=vp_t[kt][:],
                        start=(kt == 0),
                        stop=(kt == 4 * s + j),
                    )

            issue_scores(0)
            if n_k > 1:
                issue_scores(1)
            if inject is not None:
                inject()
            for kt in range(n_k):
                issue_pv(kt)
                if kt + 2 < n_k:
                    issue_scores(kt + 2)
            # normalization factors + osb (DVE work; transposes issued later)
            osb_list = []
            for j in range(4):
                rc = wrk.tile([128, 1], f32, tag="rc", name=f"rc{h}_{s}_{j}")
                nc.vector.reciprocal(rc[:], oacc[j][:, 256:257])
                osb = opool.tile([128, 256], bf16, tag="osb", name=f"osb{h}_{s}_{j}")
                nc.vector.tensor_scalar_mul(osb[:], oacc[j][:, 0:256], rc[:])
                osb_list.append(osb)
            return osb_list

        def o_trans(h, s, osb_list):
            """transpose the 4 normalized output tiles of (h, s) into otb"""
            for c in range(2):
                big = psT.tile([128, 512], bf16, tag="tp", name=f"obig{h}_{s}_{c}")
                for j in range(4):
                    nc.tensor.matmul(
                        big[:, j * 128 : (j + 1) * 128],
                        lhsT=osb_list[j][:, c * 128 : (c + 1) * 128],
                        rhs=ident_sb[:],
                        is_transpose=True,
                    )
                nc.vector.tensor_copy(otb[h * 2 + c][s][:], big[:])

        def o_proj(t):
            s, j = t // 4, t % 4
            ops = psT.tile([128, 256], f32, tag="tp", name=f"ops{t}")
            for c in range(4):
                nc.tensor.matmul(
                    ops[:],
                    lhsT=otb[c][s][:, j * 128 : (j + 1) * 128],
                    rhs=wo_sb[:, c, :],
                    start=(c == 0),
                    stop=(c == 3),
                )
            outsb = opool.tile([128, 256], f32, tag="outsb", name=f"outsb{t}")
            nc.scalar.copy(outsb[:], ops[:])
            nc.sync.dma_start(out_d[t * 128 : (t + 1) * 128, :], outsb[:])

        # pipeline: h0(s) transposes + h1(s-1) transposes + o_proj(s-1) all
        # run inside later instruction streams so their DVE inputs are ready.
        osb_mem = {}
        for s in range(NSB):

            def inj_h0(ss=s):
                if ss > 0:
                    o_trans(1, ss - 1, osb_mem[(1, ss - 1)])

            def inj_h1(ss=s):
                o_trans(0, ss, osb_mem[(0, ss)])
                if ss > 0:
                    for t in range(4 * (ss - 1), 4 * ss):
                        o_proj(t)

            osb_mem[(0, s)] = attn_superblock(0, s, inj_h0)
            osb_mem[(1, s)] = attn_superblock(1, s, inj_h1)

        # drain tail: transposes of (1, NSB-1) and final output projection
        o_trans(1, NSB - 1, osb_mem[(1, NSB - 1)])
        for t in range(4 * (NSB - 1), 4 * NSB):
            o_proj(t)

    nc.finalize()
    return nc


def _chunk2(a):
    """[256, F] -> [128, 2, F] (feature chunks on the free axis)."""
    f = a.shape[1]
    return np.ascontiguousarray(a.reshape(2, 128, f).transpose(1, 0, 2))


def _prep_core_inputs(c, x, WK_w, WK_b, WV_w, WV_b, WQ_w, WQ_b, WO_w):
    import ml_dtypes

    bf16 = ml_dtypes.bfloat16
    b, g, j = c // 4, (c // 2) % 2, c % 2
    f32 = np.float32

    xT = np.ascontiguousarray(x[:, b, :].T.astype(f32))  # [256, 2048]
    xt = _chunk2(xT)

    wk_s = WK_w[:, g * 256 : (g + 1) * 256].astype(f32)  # [256 in, 256 out]
    bk_s = WK_b[g * 256 : (g + 1) * 256].astype(f32)
    wv_s = np.ascontiguousarray(WV_w[:, g * 256 : (g + 1) * 256].astype(f32))

    col = (g * SUB + 2 * j) * 256
    wq_s = WQ_w[:, col : col + 512].astype(f32)  # both heads [256 in, 512 out]
    bq_s = WQ_b[col : col + 512].astype(f32)

    row = (g * SUB + 2 * j) * 256
    wo_s = WO_w[row : row + 512, :].astype(f32)  # [512, 256]
    wo = np.ascontiguousarray(wo_s.reshape(4, 128, 256).transpose(1, 0, 2))

    pp, ff = np.meshgrid(np.arange(128), np.arange(128), indexing="ij")
    hdr = np.concatenate(
        [
            np.ascontiguousarray(_chunk2(wk_s).reshape(128, 512)),
            xt[:, 0, 0:512],
            np.ascontiguousarray(_chunk2(wq_s).reshape(128, 1024)),
            xt[:, 1, 0:512],
        ],
        axis=1,
    )
    return {
        "xt": xt,
        "hdr": np.ascontiguousarray(hdr),
        "bkc": np.ascontiguousarray(bk_s.reshape(2, 128).T),
        "wmk": np.ascontiguousarray(
            np.repeat(wk_s.mean(axis=1, keepdims=True).reshape(2, 128).T, 2, axis=1)
        ),
        "bkm": np.full((128, 1), bk_s.mean(), dtype=f32),
        "bqc": np.ascontiguousarray(bq_s.reshape(4, 128).T),
        "wmq": np.ascontiguousarray(
            wq_s.reshape(256, 2, 256).mean(axis=2).reshape(2, 128, 2).transpose(1, 0, 2)
        ),
        "bqm": np.array(
            [[bq_s[0:256].mean(), bq_s[256:512].mean()]], dtype=f32
        ),
        "wv": _chunk2(wv_s),
        "wo": wo.astype(bf16),
        "ident": np.eye(128, dtype=bf16),
        "cmask": (pp <= ff).astype(bf16),  # keep k<=q on the diagonal tile
        "vpones": np.concatenate(
            [np.ones((128, NT, 1), dtype=bf16), np.zeros((128, NT, 1), dtype=bf16)],
            axis=2,
        ),
    }


def kernel(x, WK_w, WK_b, WV_w, WV_b, WQ_w, WQ_b, WO_w, WO_b, ln_g, ln_b, **kwargs):
    x = np.asarray(x)
    WK_w, WK_b = np.asarray(WK_w), np.asarray(WK_b)
    WV_w, WV_b = np.asarray(WV_w), np.asarray(WV_b)
    WQ_w, WQ_b = np.asarray(WQ_w), np.asarray(WQ_b)
    WO_w, WO_b = np.asarray(WO_w), np.asarray(WO_b)
    ln_g, ln_b = np.asarray(ln_g), np.asarray(ln_b)

    if not np.allclose(ln_b, 0.0):
        raise NotImplementedError("nonzero ln_b not supported by this kernel")
    if not np.allclose(ln_g, 1.0):
        raise NotImplementedError("non-unit ln_g not supported by this kernel")

    if "nc" not in _CACHE:
        _CACHE["nc"] = _build_program()
    nc = _CACHE["nc"]

    in_maps = [
        _prep_core_inputs(c, x, WK_w, WK_b, WV_w, WV_b, WQ_w, WQ_b, WO_w)
        for c in range(N_CORES)
    ]

    from concourse.bass_utils import run_bass_kernel_spmd

    res = run_bass_kernel_spmd(nc, in_maps, list(range(N_CORES)))
    results = res.results

    out = np.zeros((SEQ, BS, DIM), dtype=np.float32)
    for c in range(N_CORES):
        out[:, c // 4, :] += results[c]["out_partial"]

    # fold: WO_b plus the V-bias contribution of every head
    const_bias = WO_b.astype(np.float64).copy()
    for g in range(G):
        bv = WV_b[g * 256 : (g + 1) * 256].astype(np.float64)
        for sh in range(SUB):
            row = (g * SUB + sh) * 256
            const_bias += bv @ WO_w[row : row + 256, :].astype(np.float64)
    out += const_bias.astype(np.float32)[None, None, :]
    return out


# revision 38
# speedup vs baseline: 1.4610x; 1.0043x over previous
"""GroupedAttention Trainium2 kernel.

Problem: x[2048, 2, 256]; K/V projections to G=2 groups (head width 256),
Q projection to G*SUB=8 heads; LayerNorm on K and Q; causal softmax
attention per (b, g, sub); output projection back to 256.

Sharding: 16 (b, g, sub) heads over 8 cores -> 2 heads per core.
Core c: b = c//4, g = (c//2)%2, sub-pair j = c%2 (subs 2j, 2j+1).
The host sums the 4 partials per batch and adds a folded constant bias.

Key structure (all timings against the TRN2 cost model):
- K^T and Q^T are produced DIRECTLY by matmuls (weights stationary,
  x^T moving), eliminating every K/Q transpose on the PE.
- LayerNorm is never applied to K. Scores use raw (biased) K^T; the
  per-key factor 1/(16*std_k) folds into the Exp activation's
  per-partition scale AP, and the mean term cancels because the
  normalized Q rows sum to ~0 (ln_g == 1). Per-key mean/sumsq come from
  tiny N=1 matmuls against precomputed row-mean weight columns and a
  Square+ones-reduction, batched 4 seq-tiles per PSUM bank.
- Q IS normalized (its per-query scale sits inside the softmax):
  mean/sumsq rows are computed by M=2/M=1 matmuls into one [4,512]
  PSUM tile, converted to (mu, 1/std) rows, broadcast across partitions
  on the otherwise-idle GPSIMD engine, and applied with one
  scalar_tensor_tensor (bias add + mean subtract) plus one multiply.
- Causal structure at 128-tile granularity: score columns below the
  diagonal tile are skipped entirely (bf16 moving keeps 1 cyc/row even
  below 256 columns), PV matmuls for empty tile pairs are skipped, and
  only the diagonal 128x128 tile is masked (one shared 0/1 bf16 mask).
- A ones-column appended to V makes PSUM column 256 accumulate the
  softmax denominator for free.
- The kt loop is software-pipelined (scores two blocks ahead of PV);
  O-transposes are batched 4-per-bank with one wide PSUM->SBUF copy and
  injected, with the output projection, into the next superblock's
  instruction stream so the PE never waits on DVE chains.
- bf16 for Q^T-normalized, P, O tiles and the transpose identity
  (transposes run 1.0 cyc/row); everything accumulates in fp32 PSUM.
"""

import sys

import numpy as np

for _p in ("/opt/trn_rl_repo",):
    if _p not in sys.path:
        sys.path.insert(0, _p)

SEQ, BS, DIM = 2048, 2, 256
G, SUB = 2, 4
N_CORES = 8
LN_EPS = 1e-5
NT = SEQ // 128  # 16 seq tiles of 128
NSB = SEQ // 512  # 4 blocks of 512 (query superblocks / proj blocks)

_CACHE = {}


def _build_program():
    from contextlib import ExitStack

    import concourse.bacc as bacc
    import concourse.bass_isa as bass_isa
    import concourse.mybir as mybir
    from concourse import tile
    f32 = mybir.dt.float32
    f32r = mybir.dt.float32r
    bf16 = mybir.dt.bfloat16
    AF = mybir.ActivationFunctionType
    OP = mybir.AluOpType

    nc = bacc.Bacc("TRN2", target_bir_lowering=False, debug=False)

    xt_d = nc.dram_tensor("xt", [128, 2, SEQ], f32r, kind="ExternalInput").ap()
    hdr_d = nc.dram_tensor("hdr", [128, 2560], f32r, kind="ExternalInput").ap()
    bkc_d = nc.dram_tensor("bkc", [128, 2], f32, kind="ExternalInput").ap()
    wmk_d = nc.dram_tensor("wmk", [128, 4], f32r, kind="ExternalInput").ap()
    bkm_d = nc.dram_tensor("bkm", [128, 1], f32, kind="ExternalInput").ap()
    bqc_d = nc.dram_tensor("bqc", [128, 4], f32, kind="ExternalInput").ap()
    wmq_d = nc.dram_tensor("wmq", [128, 2, 2], f32r, kind="ExternalInput").ap()
    bqm_d = nc.dram_tensor("bqm", [1, 2], f32, kind="ExternalInput").ap()
    wv_d = nc.dram_tensor("wv", [128, 2, 256], f32r, kind="ExternalInput").ap()
    wo_d = nc.dram_tensor("wo", [128, 4, 256], bf16, kind="ExternalInput").ap()
    id_d = nc.dram_tensor("ident", [128, 128], bf16, kind="ExternalInput").ap()
    cm_d = nc.dram_tensor("cmask", [128, 128], bf16, kind="ExternalInput").ap()
    vo_d = nc.dram_tensor("vpones", [128, NT, 2], bf16, kind="ExternalInput").ap()
    out_d = nc.dram_tensor("out_partial", [SEQ, DIM], f32, kind="ExternalOutput").ap()

    r = lambda ap: ap.bitcast(f32r)

    with tile.TileContext(nc) as tc, ExitStack() as ctx:
        const = ctx.enter_context(tc.tile_pool(name="const", bufs=1))

        xt_sb = const.tile([128, 2, SEQ], f32r)
        hdr_sb = const.tile([128, 2560], f32r)
        bkc_sb = const.tile([128, 2], f32)
        wmk_sb = const.tile([128, 4], f32r)
        bkm_sb = const.tile([128, 1], f32)
        bqc_sb = const.tile([128, 4], f32)
        wmq_sb = const.tile([128, 2, 2], f32r)
        bqm_sb = const.tile([1, 2], f32)
        wv_sb = const.tile([128, 2, 256], f32r)
        wo_sb = const.tile([128, 4, 256], bf16)
        ident_sb = const.tile([128, 128], bf16)
        tmask_sb = const.tile([128, 128], bf16)
        onescol_sb = const.tile([128, 2], f32)
        epsk_sb = const.tile([128, 1], f32)
        epsq_sb = const.tile([1, 1], f32)

        # persistent SBUF activations
        ktb = [
            [const.tile([128, 512], bf16, name=f"ktb{f}_{b}") for b in range(NSB)]
            for f in range(2)
        ]
        qtn = [
            [const.tile([128, 512], bf16, name=f"qtn{fc}_{b}") for b in range(NSB)]
            for fc in range(4)
        ]
        vp_t = [const.tile([128, 258], bf16, name=f"vpt{t}") for t in range(NT)]
        rk16b = [const.tile([128, 8], f32, name=f"rk16b{b}") for b in range(NSB)]
        otb = [
            [const.tile([128, 512], bf16, name=f"otb{c}_{s}") for s in range(NSB)]
            for c in range(4)
        ]

        nc.gpsimd.memset(onescol_sb[:], 1.0)
        nc.gpsimd.memset(epsk_sb[:], 256.0 * LN_EPS)
        nc.gpsimd.memset(epsq_sb[:], LN_EPS)
        nc.sync.dma_start(hdr_sb[:, 0:1024], hdr_d[:, 0:1024])
        nc.sync.dma_start(hdr_sb[:, 1024:1536], hdr_d[:, 1024:1536])
        nc.sync.dma_start(hdr_sb[:, 1536:2560], hdr_d[:, 1536:2560])
        nc.sync.dma_start(bkc_sb[:], bkc_d[:])
        nc.sync.dma_start(wmk_sb[:], wmk_d[:])
        nc.sync.dma_start(bkm_sb[:], bkm_d[:])
        nc.sync.dma_start(bqc_sb[:], bqc_d[:])
        nc.sync.dma_start(wmq_sb[:], wmq_d[:])
        nc.sync.dma_start(bqm_sb[:], bqm_d[:])
        nc.sync.dma_start(wv_sb[:], wv_d[:])
        nc.sync.dma_start(xt_sb[:, :, 512:1024], xt_d[:, :, 512:1024])
        nc.sync.dma_start(xt_sb[:, :, 1024:1536], xt_d[:, :, 1024:1536])
        nc.sync.dma_start(xt_sb[:, :, 1536:2048], xt_d[:, :, 1536:2048])
        nc.sync.dma_start(ident_sb[:], id_d[:])
        nc.sync.dma_start(tmask_sb[:], cm_d[:])
        for t in range(NT):
            nc.sync.dma_start(vp_t[t][:, 256:258], vo_d[:, t, :])
        nc.sync.dma_start(wo_sb[:], wo_d[:])


        def wk_v(c, f):
            return hdr_sb[:, c * 256 + f * 128 : c * 256 + (f + 1) * 128]

        def wq_v(c, lo, hi):
            return hdr_sb[:, 1536 + c * 512 + lo : 1536 + c * 512 + hi]

        def xt_v(c, lo, hi):
            if hi <= 512:
                base = 512 + c * 512
                return hdr_sb[:, base + lo : base + hi]
            return xt_sb[:, c, lo:hi]

        wrk = ctx.enter_context(tc.tile_pool(name="wrk", bufs=3))
        ppool = ctx.enter_context(tc.tile_pool(name="ppool", bufs=6))
        opool = ctx.enter_context(tc.tile_pool(name="opool", bufs=6))

        # ======== projection phase (pools scoped; close before attention) ====
        with tc.tile_pool(name="projp", bufs=3, space="PSUM") as projp, \
             tc.tile_pool(name="tinyp", bufs=1, space="PSUM") as tinyp, \
             tc.tile_pool(name="rowp", bufs=1, space="PSUM") as rowp:
            for B in range(NSB):
                sl = slice(B * 512, (B + 1) * 512)
                # --- K^T chunks: biased SBUF copy + biased square (from PSUM)
                ktsq = []
                for f in range(2):
                    kps = projp.tile([128, 512], f32, tag="proj", name=f"kps{f}_{B}")
                    for c in range(2):
                        nc.tensor.matmul(
                            kps[:],
                            lhsT=r(wk_v(c, f)),
                            rhs=r(xt_v(c, B * 512, (B + 1) * 512)),
                            start=(c == 0),
                            stop=(c == 1),
                        )
                    ksq = wrk.tile([128, 512], f32r, tag=f"ksq{f}", bufs=2,
                                   name=f"ksq{f}_{B}")
                    nc.scalar.activation(
                        ksq[:], kps[:], AF.Square, bias=bkc_sb[:, f : f + 1]
                    )
                    nc.vector.tensor_scalar_add(
                        ktb[f][B][:], kps[:], scalar1=bkc_sb[:, f : f + 1]
                    )
                    ktsq.append(ksq)
                # --- Q^T chunks: raw in PSUM until normalize; biased square
                qps_l, qtsq = [], []
                for fc in range(4):
                    qps = projp.tile([128, 512], f32, tag="proj", name=f"qps{fc}_{B}")
                    for c in range(2):
                        nc.tensor.matmul(
                            qps[:],
                            lhsT=r(wq_v(c, fc * 128, (fc + 1) * 128)),
                            rhs=r(xt_v(c, B * 512, (B + 1) * 512)),
                            start=(c == 0),
                            stop=(c == 1),
                        )
                    qsq = wrk.tile([128, 512], f32r, tag=f"qsq{fc}", bufs=2,
                                   name=f"qsq{fc}_{B}")
                    nc.scalar.activation(
                        qsq[:], qps[:], AF.Square, bias=bqc_sb[:, fc : fc + 1]
                    )
                    qps_l.append(qps)
                    qtsq.append(qsq)
                # --- V tiles ---
                for t in range(4 * B, 4 * B + 4):
                    vps = projp.tile([128, 256], f32, tag="proj", name=f"vps{t}")
                    for c in range(2):
                        nc.tensor.matmul(
                            vps[:],
                            lhsT=r(xt_v(c, t * 128, (t + 1) * 128)),
                            rhs=r(wv_sb[:, c, :]),
                            start=(c == 0),
                            stop=(c == 1),
                        )
                    nc.scalar.copy(vp_t[t][:, 0:256], vps[:])
                # --- K per-key stats: mu (cols 0:4) and sumsq (cols 4:8) ---
                tiny = tinyp.tile([128, 16], f32, tag="tiny", name=f"tiny{B}")
                for i, t in enumerate(range(4 * B, 4 * B + 4)):
                    for c in range(2):
                        nc.tensor.matmul(
                            tiny[:, 2 * i : 2 * i + 2],
                            lhsT=r(xt_v(c, t * 128, (t + 1) * 128)),
                            rhs=wmk_sb[:, 2 * c : 2 * c + 2],
                            start=(c == 0),
                            stop=(c == 1),
                        )
                    for f in range(2):
                        nc.tensor.matmul(
                            tiny[:, 8 + 2 * i : 10 + 2 * i],
                            lhsT=r(ktsq[f][:, i * 128 : (i + 1) * 128]),
                            rhs=r(onescol_sb[:, 0:2]),
                            start=(f == 0),
                            stop=(f == 1),
                        )
                # --- Q row stats, one bank per head: mu at partition 0,
                # sumsq at partition 32 (matmul base must be 0/32/64)
                mur_h = [
                    rowp.tile([1, 512], f32, tag=f"mur{h}", name=f"mur{h}_{B}")[:]
                    for h in range(2)
                ]
                for h in range(2):
                    for c in range(2):
                        nc.tensor.matmul(
                            mur_h[h],
                            lhsT=wmq_sb[:, c, h : h + 1],
                            rhs=r(xt_v(c, B * 512, (B + 1) * 512)),
                            start=(c == 0),
                            stop=(c == 1),
                        )
                sqr_h = [
                    rowp.tile([1, 512], f32, tag=f"sqr{h}", name=f"sqr{h}_{B}")[:]
                    for h in range(2)
                ]
                for h in range(2):
                    for c in range(2):
                        nc.tensor.matmul(
                            sqr_h[h],
                            lhsT=r(onescol_sb[:, 0:1]),
                            rhs=r(qtsq[2 * h + c][:]),
                            start=(c == 0),
                            stop=(c == 1),
                        )
                # --- K stats -> rk16 (batched over the 4 seq tiles) ---
                mu2 = wrk.tile([128, 8], f32, tag="mu2", name=f"mu2_{B}")
                nc.scalar.activation(mu2[:], tiny[:, 0:8], AF.Square, bias=bkm_sb[:])
                v256 = wrk.tile([128, 8], f32, tag="v256", name=f"v256_{B}")
                nc.vector.scalar_tensor_tensor(
                    v256[:], mu2[:], -256.0, tiny[:, 8:16],
                    op0=OP.mult, op1=OP.add,
                )
                std16 = wrk.tile([128, 8], f32, tag="std16", name=f"std16_{B}")
                nc.scalar.activation(std16[:], v256[:], AF.Sqrt, bias=epsk_sb[:])
                nc.vector.reciprocal(rk16b[B][:], std16[:])
                # --- Q row stats -> (mu_biased, 1/std) rows + broadcasts ---
                mub = [None, None]
                rqb = [None, None]
                for h in range(2):
                    murow = wrk.tile([1, 512], f32, tag=f"murow{h}", bufs=1,
                                     name=f"murow{h}_{B}")
                    nc.vector.tensor_scalar_add(
                        murow[:], mur_h[h],
                        scalar1=bqm_sb[0:1, h : h + 1],
                    )
                    mu2r = wrk.tile([1, 512], f32, tag=f"mu2r{h}", bufs=1,
                                    name=f"mu2r{h}_{B}")
                    nc.scalar.activation(mu2r[:], murow[:], AF.Square)
                    v256r = wrk.tile([1, 512], f32, tag=f"v256r{h}", bufs=1,
                                     name=f"v256r{h}_{B}")
                    nc.vector.scalar_tensor_tensor(
                        v256r[:], mu2r[:], -256.0, sqr_h[h],
                        op0=OP.mult, op1=OP.add,
                    )
                    stdr = wrk.tile([1, 512], f32, tag=f"stdr{h}", bufs=1,
                                    name=f"stdr{h}_{B}")
                    nc.scalar.activation(
                        stdr[:], v256r[:], AF.Sqrt, bias=epsq_sb[:],
                        scale=1.0 / 256.0,
                    )
                    rqrow = wrk.tile([1, 512], f32, tag=f"rqrow{h}", bufs=1,
                                     name=f"rqrow{h}_{B}")
                    nc.vector.reciprocal(rqrow[:], stdr[:])
                    mub[h] = wrk.tile([128, 512], f32, tag=f"mub{h}", bufs=1,
                                      name=f"mub{h}_{B}")
                    nc.gpsimd.partition_broadcast(mub[h][:], murow[:])
                    rqb[h] = wrk.tile([128, 512], f32, tag=f"rqb{h}", bufs=1,
                                      name=f"rqb{h}_{B}")
                    nc.gpsimd.partition_broadcast(rqb[h][:], rqrow[:])
                # --- normalize Q: ((raw + bias) - mu) * (1/std) -> bf16 ---
                for fc in range(4):
                    h = fc // 2
                    qtmp = wrk.tile([128, 512], f32, tag=f"qtmp{fc % 2}",
                                    name=f"qtmp{fc}_{B}")
                    nc.vector.scalar_tensor_tensor(
                        qtmp[:], qps_l[fc][:], bqc_sb[:, fc : fc + 1], mub[h][:],
                        op0=OP.add, op1=OP.subtract,
                    )
                    eng = nc.vector if fc % 2 == 0 else nc.gpsimd
                    eng.tensor_mul(qtn[fc][B][:], qtmp[:], rqb[h][:])

        # ======== attention phase ========
        psA = ctx.enter_context(tc.tile_pool(name="psA", bufs=2, space="PSUM"))
        psB = ctx.enter_context(tc.tile_pool(name="psB", bufs=1, space="PSUM"))
        psT = ctx.enter_context(tc.tile_pool(name="psT", bufs=2, space="PSUM"))

        def attn_superblock(h, s, inject=None):
            n_k = 4 * (s + 1)
            oacc = [
                psB.tile([128, 258], f32, tag=f"oacc{j}", name=f"oacc{h}_{s}_{j}")
                for j in range(4)
            ]
            p_tiles = [None] * n_k

            def issue_scores(kt):
                d = kt - 4 * s  # >= 0 on the diagonal region
                qoff = 0 if d <= 0 else d * 128  # bf16 moving: 1 cyc/row anyway
                st = psA.tile([128, 512], f32, tag="mm512", name=f"st{h}_{s}_{kt}")
                for c in range(2):
                    nc.tensor.matmul(
                        st[:, qoff:512],
                        lhsT=ktb[c][kt // 4][:, (kt % 4) * 128 : (kt % 4 + 1) * 128],
                        rhs=qtn[h * 2 + c][s][:, qoff:512],
                        start=(c == 0),
                        stop=(c == 1),
                    )
                p = ppool.tile([128, 512], bf16, tag="p", name=f"p{h}_{s}_{kt}")
                nc.scalar.activation(
                    p[:, qoff:512], st[:, qoff:512], AF.Exp,
                    scale=rk16b[kt // 4][:, 2 * (kt % 4) : 2 * (kt % 4) + 1],
                )
                if d >= 0:
                    nc.vector.tensor_mul(
                        p[:, d * 128 : (d + 1) * 128],
                        p[:, d * 128 : (d + 1) * 128],
                        tmask_sb[:],
                    )
                p_tiles[kt] = p

            def issue_pv(kt):
                d = kt - 4 * s
                p = p_tiles[kt]
                for j in range(max(d, 0), 4):
                    nc.tensor.matmul(
                        oacc[j][:],
                        lhsT=p[:, j * 128 : (j + 1) * 128],
                        rhs=vp_t[kt][:],
                        start=(kt == 0),
                        stop=(kt == 4 * s + j),
                    )

            issue_scores(0)
            if n_k > 1:
                issue_scores(1)
            if inject is not None:
                inject()
            for kt in range(n_k):
                issue_pv(kt)
                if kt + 2 < n_k:
                    issue_scores(kt + 2)
            # normalization factors + osb (DVE work; transposes issued later)
            osb_list = []
            for j in range(4):
                rc = wrk.tile([128, 1], f32, tag="rc", name=f"rc{h}_{s}_{j}")
                nc.vector.reciprocal(rc[:], oacc[j][:, 256:257])
                osb = opool.tile([128, 256], bf16, tag="osb", name=f"osb{h}_{s}_{j}")
                nc.vector.tensor_scalar_mul(osb[:], oacc[j][:, 0:256], rc[:])
                osb_list.append(osb)
            return osb_list

        def o_trans(h, s, osb_list):
            """transpose the 4 normalized output tiles of (h, s) into otb"""
            for c in range(2):
                big = psT.tile([128, 512], bf16, tag="tp", name=f"obig{h}_{s}_{c}")
                for j in range(4):
                    nc.tensor.matmul(
                        big[:, j * 128 : (j + 1) * 128],
                        lhsT=osb_list[j][:, c * 128 : (c + 1) * 128],
                        rhs=ident_sb[:],
                        is_transpose=True,
                    )
                nc.vector.tensor_copy(otb[h * 2 + c][s][:], big[:])

        def o_proj(t):
            s, j = t // 4, t % 4
            ops = psT.tile([128, 256], f32, tag="tp", name=f"ops{t}")
            for c in range(4):
                nc.tensor.matmul(
                    ops[:],
                    lhsT=otb[c][s][:, j * 128 : (j + 1) * 128],
                    rhs=wo_sb[:, c, :],
                    start=(c == 0),
                    stop=(c == 3),
                )
            outsb = opool.tile([128, 256], f32, tag="outsb", name=f"outsb{t}")
            nc.scalar.copy(outsb[:], ops[:])
            nc.sync.dma_start(out_d[t * 128 : (t + 1) * 128, :], outsb[:])

        # pipeline: h0(s) transposes + h1(s-1) transposes + o_proj(s-1) all
        # run inside later instruction streams so their DVE inputs are ready.
        osb_mem = {}
        for s in range(NSB):

            def inj_h0(ss=s):
                if ss > 0:
                    o_trans(1, ss - 1, osb_mem[(1, ss - 1)])

            def inj_h1(ss=s):
                o_trans(0, ss, osb_mem[(0, ss)])
                if ss > 0:
                    for t in range(4 * (ss - 1), 4 * ss):
                        o_proj(t)

            osb_mem[(0, s)] = attn_superblock(0, s, inj_h0)
            osb_mem[(1, s)] = attn_superblock(1, s, inj_h1)

        # drain tail, j-granular: transpose/copy/project/store per query tile
        s_last = NSB - 1
        osb_l = osb_mem[(1, s_last)]
        for j in range(4):
            jb = psT.tile([128, 256], bf16, tag="tp", name=f"jb{j}")
            for c in range(2):
                nc.tensor.matmul(
                    jb[:, c * 128 : (c + 1) * 128],
                    lhsT=osb_l[j][:, c * 128 : (c + 1) * 128],
                    rhs=ident_sb[:],
                    is_transpose=True,
                )
            for c in range(2):
                nc.vector.tensor_copy(
                    otb[2 + c][s_last][:, j * 128 : (j + 1) * 128],
                    jb[:, c * 128 : (c + 1) * 128],
                )
            o_proj(4 * s_last + j)

    nc.finalize()
    return nc


def _chunk2(a):
    """[256, F] -> [128, 2, F] (feature chunks on the free axis)."""
    f = a.shape[1]
    return np.ascontiguousarray(a.reshape(2, 128, f).transpose(1, 0, 2))


def _prep_core_inputs(c, x, WK_w, WK_b, WV_w, WV_b, WQ_w, WQ_b, WO_w):
    import ml_dtypes

    bf16 = ml_dtypes.bfloat16
    b, g, j = c // 4, (c // 2) % 2, c % 2
    f32 = np.float32

    xT = np.ascontiguousarray(x[:, b, :].T.astype(f32))  # [256, 2048]
    xt = _chunk2(xT)

    wk_s = WK_w[:, g * 256 : (g + 1) * 256].astype(f32)  # [256 in, 256 out]
    bk_s = WK_b[g * 256 : (g + 1) * 256].astype(f32)
    wv_s = np.ascontiguousarray(WV_w[:, g * 256 : (g + 1) * 256].astype(f32))

    col = (g * SUB + 2 * j) * 256
    wq_s = WQ_w[:, col : col + 512].astype(f32)  # both heads [256 in, 512 out]
    bq_s = WQ_b[col : col + 512].astype(f32)

    row = (g * SUB + 2 * j) * 256
    wo_s = WO_w[row : row + 512, :].astype(f32)  # [512, 256]
    wo = np.ascontiguousarray(wo_s.reshape(4, 128, 256).transpose(1, 0, 2))

    pp, ff = np.meshgrid(np.arange(128), np.arange(128), indexing="ij")
    hdr = np.concatenate(
        [
            np.ascontiguousarray(_chunk2(wk_s).reshape(128, 512)),
            xt[:, 0, 0:512],
            xt[:, 1, 0:512],
            np.ascontiguousarray(_chunk2(wq_s).reshape(128, 1024)),
        ],
        axis=1,
    )
    return {
        "xt": xt,
        "hdr": np.ascontiguousarray(hdr),
        "bkc": np.ascontiguousarray(bk_s.reshape(2, 128).T),
        "wmk": np.ascontiguousarray(
            np.repeat(wk_s.mean(axis=1, keepdims=True).reshape(2, 128).T, 2, axis=1)
        ),
        "bkm": np.full((128, 1), bk_s.mean(), dtype=f32),
        "bqc": np.ascontiguousarray(bq_s.reshape(4, 128).T),
        "wmq": np.ascontiguousarray(
            wq_s.reshape(256, 2, 256).mean(axis=2).reshape(2, 128, 2).transpose(1, 0, 2)
        ),
        "bqm": np.array(
            [[bq_s[0:256].mean(), bq_s[256:512].mean()]], dtype=f32
        ),
        "wv": _chunk2(wv_s),
        "wo": wo.astype(bf16),
        "ident": np.eye(128, dtype=bf16),
        "cmask": (pp <= ff).astype(bf16),  # keep k<=q on the diagonal tile
        "vpones": np.concatenate(
            [np.ones((128, NT, 1), dtype=bf16), np.zeros((128, NT, 1), dtype=bf16)],
            axis=2,
        ),
    }


def kernel(x, WK_w, WK_b, WV_w, WV_b, WQ_w, WQ_b, WO_w, WO_b, ln_g, ln_b, **kwargs):
    x = np.asarray(x)
    WK_w, WK_b = np.asarray(WK_w), np.asarray(WK_b)
    WV_w, WV_b = np.asarray(WV_w), np.asarray(WV_b)
    WQ_w, WQ_b = np.asarray(WQ_w), np.asarray(WQ_b)
    WO_w, WO_b = np.asarray(WO_w), np.asarray(WO_b)
    ln_g, ln_b = np.asarray(ln_g), np.asarray(ln_b)

    if not np.allclose(ln_b, 0.0):
        raise NotImplementedError("nonzero ln_b not supported by this kernel")
    if not np.allclose(ln_g, 1.0):
        raise NotImplementedError("non-unit ln_g not supported by this kernel")

    if "nc" not in _CACHE:
        _CACHE["nc"] = _build_program()
    nc = _CACHE["nc"]

    in_maps = [
        _prep_core_inputs(c, x, WK_w, WK_b, WV_w, WV_b, WQ_w, WQ_b, WO_w)
        for c in range(N_CORES)
    ]

    from concourse.bass_utils import run_bass_kernel_spmd

    res = run_bass_kernel_spmd(nc, in_maps, list(range(N_CORES)))
    results = res.results

    out = np.zeros((SEQ, BS, DIM), dtype=np.float32)
    for c in range(N_CORES):
        out[:, c // 4, :] += results[c]["out_partial"]

    # fold: WO_b plus the V-bias contribution of every head
    const_bias = WO_b.astype(np.float64).copy()
    for g in range(G):
        bv = WV_b[g * 256 : (g + 1) * 256].astype(np.float64)
        for sh in range(SUB):
            row = (g * SUB + sh) * 256
            const_bias += bv @ WO_w[row : row + 256, :].astype(np.float64)
    out += const_bias.astype(np.float32)[None, None, :]
    return out


# revision 50
# speedup vs baseline: 1.5741x; 1.0774x over previous
"""GroupedAttention Trainium2 kernel.

Problem: x[2048, 2, 256]; K/V projections to G=2 groups (head width 256),
Q projection to G*SUB=8 heads; LayerNorm on K and Q; causal softmax
attention per (b, g, sub); output projection back to 256.

Sharding: 16 (b, g, sub) heads over 8 cores -> 2 heads per core.
Core c: b = c//4, g = (c//2)%2, sub-pair j = c%2 (subs 2j, 2j+1).
The host sums the 4 partials per batch and adds a folded constant bias.

Key structure (all timings against the TRN2 cost model):
- K^T and Q^T are produced DIRECTLY by matmuls (weights stationary,
  x^T moving), eliminating every K/Q transpose on the PE.
- LayerNorm is never applied to K. Scores use raw (biased) K^T; the
  per-key factor 1/(16*std_k) folds into the Exp activation's
  per-partition scale AP, and the mean term cancels because the
  normalized Q rows sum to ~0 (ln_g == 1). Per-key mean/sumsq come from
  tiny N=1 matmuls against precomputed row-mean weight columns and a
  Square+ones-reduction, batched 4 seq-tiles per PSUM bank.
- Q IS normalized (its per-query scale sits inside the softmax):
  mean/sumsq rows are computed by M=2/M=1 matmuls into one [4,512]
  PSUM tile, converted to (mu, 1/std) rows, broadcast across partitions
  on the otherwise-idle GPSIMD engine, and applied with one
  scalar_tensor_tensor (bias add + mean subtract) plus one multiply.
- Causal structure at 128-tile granularity: score columns below the
  diagonal tile are skipped entirely (bf16 moving keeps 1 cyc/row even
  below 256 columns), PV matmuls for empty tile pairs are skipped, and
  only the diagonal 128x128 tile is masked (one shared 0/1 bf16 mask).
- A ones-column appended to V makes PSUM column 256 accumulate the
  softmax denominator for free.
- The kt loop is software-pipelined (scores two blocks ahead of PV);
  O-transposes are batched 4-per-bank with one wide PSUM->SBUF copy and
  injected, with the output projection, into the next superblock's
  instruction stream so the PE never waits on DVE chains.
- bf16 for Q^T-normalized, P, O tiles and the transpose identity
  (transposes run 1.0 cyc/row); everything accumulates in fp32 PSUM.
"""

import sys

import numpy as np

for _p in ("/opt/trn_rl_repo",):
    if _p not in sys.path:
        sys.path.insert(0, _p)

SEQ, BS, DIM = 2048, 2, 256
G, SUB = 2, 4
N_CORES = 8
LN_EPS = 1e-5
NT = SEQ // 128  # 16 seq tiles of 128
NSB = SEQ // 512  # 4 blocks of 512 (query superblocks / proj blocks)

_CACHE = {}


def _build_program():
    from contextlib import ExitStack

    import concourse.bacc as bacc
    import concourse.bass_isa as bass_isa
    import concourse.mybir as mybir
    from concourse import tile
    f32 = mybir.dt.float32
    f32r = mybir.dt.float32r
    bf16 = mybir.dt.bfloat16
    AF = mybir.ActivationFunctionType
    OP = mybir.AluOpType

    nc = bacc.Bacc("TRN2", target_bir_lowering=False, debug=False)

    xt_d = nc.dram_tensor("xt", [128, 2, SEQ], bf16, kind="ExternalInput").ap()
    hdr_d = nc.dram_tensor("hdr", [128, 2560], bf16, kind="ExternalInput").ap()
    bkc_d = nc.dram_tensor("bkc", [128, 2], f32, kind="ExternalInput").ap()
    wmk_d = nc.dram_tensor("wmk", [128, 4], bf16, kind="ExternalInput").ap()
    bkm_d = nc.dram_tensor("bkm", [128, 1], f32, kind="ExternalInput").ap()
    bqc_d = nc.dram_tensor("bqc", [128, 4], f32, kind="ExternalInput").ap()
    wmq_d = nc.dram_tensor("wmq", [128, 2, 2], bf16, kind="ExternalInput").ap()
    bqm_d = nc.dram_tensor("bqm", [1, 2], f32, kind="ExternalInput").ap()
    wv_d = nc.dram_tensor("wv", [128, 2, 256], bf16, kind="ExternalInput").ap()
    wo_d = nc.dram_tensor("wo", [128, 4, 256], bf16, kind="ExternalInput").ap()
    id_d = nc.dram_tensor("ident", [128, 128], bf16, kind="ExternalInput").ap()
    cm_d = nc.dram_tensor("cmask", [128, 128], bf16, kind="ExternalInput").ap()
    vo_d = nc.dram_tensor("vpones", [128, NT, 2], bf16, kind="ExternalInput").ap()
    out_d = nc.dram_tensor("out_partial", [SEQ, DIM], f32, kind="ExternalOutput").ap()

    r = lambda ap: ap.bitcast(f32r)

    with tile.TileContext(nc) as tc, ExitStack() as ctx:
        const = ctx.enter_context(tc.tile_pool(name="const", bufs=1))

        xt_sb = const.tile([128, 2, SEQ], bf16)
        hdr_sb = const.tile([128, 2560], bf16)
        bkc_sb = const.tile([128, 2], f32)
        wmk_sb = const.tile([128, 4], bf16)
        bkm_sb = const.tile([128, 1], f32)
        bqc_sb = const.tile([128, 4], f32)
        wmq_sb = const.tile([128, 2, 2], bf16)
        bqm_sb = const.tile([1, 2], f32)
        wv_sb = const.tile([128, 2, 256], bf16)
        wo_sb = const.tile([128, 4, 256], bf16)
        ident_sb = const.tile([128, 128], bf16)
        tmask_sb = const.tile([128, 128], bf16)
        onescol_sb = const.tile([128, 2], f32)
        epsk_sb = const.tile([128, 1], f32)
        epsq_sb = const.tile([1, 1], f32)

        # persistent SBUF activations
        ktb = [
            [const.tile([128, 512], bf16, name=f"ktb{f}_{b}") for b in range(NSB)]
            for f in range(2)
        ]
        qtn = [
            [const.tile([128, 512], bf16, name=f"qtn{fc}_{b}") for b in range(NSB)]
            for fc in range(4)
        ]
        vp_t = [const.tile([128, 258], bf16, name=f"vpt{t}") for t in range(NT)]
        rk16b = [const.tile([128, 8], f32, name=f"rk16b{b}") for b in range(NSB)]
        otb = [
            [const.tile([128, 512], bf16, name=f"otb{c}_{s}") for s in range(NSB)]
            for c in range(4)
        ]

        nc.gpsimd.memset(onescol_sb[:], 1.0)
        nc.gpsimd.memset(epsk_sb[:], 256.0 * LN_EPS)
        nc.gpsimd.memset(epsq_sb[:], LN_EPS)
        nc.sync.dma_start(hdr_sb[:, 0:1024], hdr_d[:, 0:1024])
        nc.sync.dma_start(hdr_sb[:, 1024:1536], hdr_d[:, 1024:1536])
        nc.sync.dma_start(hdr_sb[:, 1536:2560], hdr_d[:, 1536:2560])
        nc.sync.dma_start(bkc_sb[:], bkc_d[:])
        nc.sync.dma_start(wmk_sb[:], wmk_d[:])
        nc.sync.dma_start(bkm_sb[:], bkm_d[:])
        nc.sync.dma_start(bqc_sb[:], bqc_d[:])
        nc.sync.dma_start(wmq_sb[:], wmq_d[:])
        nc.sync.dma_start(bqm_sb[:], bqm_d[:])
        nc.sync.dma_start(wv_sb[:], wv_d[:])
        nc.sync.dma_start(xt_sb[:, :, 512:1024], xt_d[:, :, 512:1024])
        nc.sync.dma_start(xt_sb[:, :, 1024:1536], xt_d[:, :, 1024:1536])
        nc.sync.dma_start(xt_sb[:, :, 1536:2048], xt_d[:, :, 1536:2048])
        nc.sync.dma_start(ident_sb[:], id_d[:])
        nc.sync.dma_start(tmask_sb[:], cm_d[:])
        for t in range(NT):
            nc.sync.dma_start(vp_t[t][:, 256:258], vo_d[:, t, :])
        nc.sync.dma_start(wo_sb[:], wo_d[:])


        def wk_v(c, f):
            return hdr_sb[:, c * 256 + f * 128 : c * 256 + (f + 1) * 128]

        def wq_v(c, lo, hi):
            return hdr_sb[:, 1536 + c * 512 + lo : 1536 + c * 512 + hi]

        def xt_v(c, lo, hi):
            if hi <= 512:
                base = 512 + c * 512
                return hdr_sb[:, base + lo : base + hi]
            return xt_sb[:, c, lo:hi]

        wrk = ctx.enter_context(tc.tile_pool(name="wrk", bufs=3))
        ppool = ctx.enter_context(tc.tile_pool(name="ppool", bufs=6))
        opool = ctx.enter_context(tc.tile_pool(name="opool", bufs=6))

        # ======== projection phase (pools scoped; close before attention) ====
        with tc.tile_pool(name="projp", bufs=3, space="PSUM") as projp, \
             tc.tile_pool(name="tinyp", bufs=1, space="PSUM") as tinyp, \
             tc.tile_pool(name="rowp", bufs=1, space="PSUM") as rowp:
            for B in range(NSB):
                sl = slice(B * 512, (B + 1) * 512)
                # --- K^T chunks: biased SBUF copy + biased square (from PSUM)
                ktsq = []
                for f in range(2):
                    kps = projp.tile([128, 512], f32, tag="proj", name=f"kps{f}_{B}")
                    for c in range(2):
                        nc.tensor.matmul(
                            kps[:],
                            lhsT=wk_v(c, f),
                            rhs=xt_v(c, B * 512, (B + 1) * 512),
                            start=(c == 0),
                            stop=(c == 1),
                        )
                    ksq = wrk.tile([128, 512], f32r, tag=f"ksq{f}", bufs=2,
                                   name=f"ksq{f}_{B}")
                    nc.scalar.activation(
                        ksq[:], kps[:], AF.Square, bias=bkc_sb[:, f : f + 1]
                    )
                    nc.vector.tensor_scalar_add(
                        ktb[f][B][:], kps[:], scalar1=bkc_sb[:, f : f + 1]
                    )
                    ktsq.append(ksq)
                # --- Q^T chunks: raw in PSUM until normalize; biased square
                qps_l, qtsq = [], []
                for fc in range(4):
                    qps = projp.tile([128, 512], f32, tag="proj", name=f"qps{fc}_{B}")
                    for c in range(2):
                        nc.tensor.matmul(
                            qps[:],
                            lhsT=wq_v(c, fc * 128, (fc + 1) * 128),
                            rhs=xt_v(c, B * 512, (B + 1) * 512),
                            start=(c == 0),
                            stop=(c == 1),
                        )
                    qsq = wrk.tile([128, 512], f32r, tag=f"qsq{fc}", bufs=2,
                                   name=f"qsq{fc}_{B}")
                    nc.scalar.activation(
                        qsq[:], qps[:], AF.Square, bias=bqc_sb[:, fc : fc + 1]
                    )
                    qps_l.append(qps)
                    qtsq.append(qsq)
                # --- V tiles ---
                for t in range(4 * B, 4 * B + 4):
                    vps = projp.tile([128, 256], f32, tag="proj", name=f"vps{t}")
                    for c in range(2):
                        nc.tensor.matmul(
                            vps[:],
                            lhsT=xt_v(c, t * 128, (t + 1) * 128),
                            rhs=wv_sb[:, c, :],
                            start=(c == 0),
                            stop=(c == 1),
                        )
                    nc.scalar.copy(vp_t[t][:, 0:256], vps[:])
                # --- K per-key stats: mu (cols 0:4) and sumsq (cols 4:8) ---
                tiny = tinyp.tile([128, 16], f32, tag="tiny", name=f"tiny{B}")
                for i, t in enumerate(range(4 * B, 4 * B + 4)):
                    for c in range(2):
                        nc.tensor.matmul(
                            tiny[:, 2 * i : 2 * i + 2],
                            lhsT=xt_v(c, t * 128, (t + 1) * 128),
                            rhs=wmk_sb[:, 2 * c : 2 * c + 2],
                            start=(c == 0),
                            stop=(c == 1),
                        )
                    for f in range(2):
                        nc.tensor.matmul(
                            tiny[:, 8 + 2 * i : 10 + 2 * i],
                            lhsT=r(ktsq[f][:, i * 128 : (i + 1) * 128]),
                            rhs=r(onescol_sb[:, 0:2]),
                            start=(f == 0),
                            stop=(f == 1),
                        )
                # --- Q row stats, one bank per head: mu at partition 0,
                # sumsq at partition 32 (matmul base must be 0/32/64)
                mur_h = [
                    rowp.tile([1, 512], f32, tag=f"mur{h}", name=f"mur{h}_{B}")[:]
                    for h in range(2)
                ]
                for h in range(2):
                    for c in range(2):
                        nc.tensor.matmul(
                            mur_h[h],
                            lhsT=wmq_sb[:, c, h : h + 1],
                            rhs=xt_v(c, B * 512, (B + 1) * 512),
                            start=(c == 0),
                            stop=(c == 1),
                        )
                sqr_h = [
                    rowp.tile([1, 512], f32, tag=f"sqr{h}", name=f"sqr{h}_{B}")[:]
                    for h in range(2)
                ]
                for h in range(2):
                    for c in range(2):
                        nc.tensor.matmul(
                            sqr_h[h],
                            lhsT=r(onescol_sb[:, 0:1]),
                            rhs=r(qtsq[2 * h + c][:]),
                            start=(c == 0),
                            stop=(c == 1),
                        )
                # --- K stats -> rk16 (batched over the 4 seq tiles) ---
                mu2 = wrk.tile([128, 8], f32, tag="mu2", name=f"mu2_{B}")
                nc.scalar.activation(mu2[:], tiny[:, 0:8], AF.Square, bias=bkm_sb[:])
                v256 = wrk.tile([128, 8], f32, tag="v256", name=f"v256_{B}")
                nc.vector.scalar_tensor_tensor(
                    v256[:], mu2[:], -256.0, tiny[:, 8:16],
                    op0=OP.mult, op1=OP.add,
                )
                std16 = wrk.tile([128, 8], f32, tag="std16", name=f"std16_{B}")
                nc.scalar.activation(std16[:], v256[:], AF.Sqrt, bias=epsk_sb[:])
                nc.vector.reciprocal(rk16b[B][:], std16[:])
                # --- Q row stats -> (mu_biased, 1/std) rows + broadcasts ---
                mub = [None, None]
                rqb = [None, None]
                for h in range(2):
                    murow = wrk.tile([1, 512], f32, tag=f"murow{h}", bufs=1,
                                     name=f"murow{h}_{B}")
                    nc.vector.tensor_scalar_add(
                        murow[:], mur_h[h],
                        scalar1=bqm_sb[0:1, h : h + 1],
                    )
                    mu2r = wrk.tile([1, 512], f32, tag=f"mu2r{h}", bufs=1,
                                    name=f"mu2r{h}_{B}")
                    nc.scalar.activation(mu2r[:], murow[:], AF.Square)
                    v256r = wrk.tile([1, 512], f32, tag=f"v256r{h}", bufs=1,
                                     name=f"v256r{h}_{B}")
                    nc.vector.scalar_tensor_tensor(
                        v256r[:], mu2r[:], -256.0, sqr_h[h],
                        op0=OP.mult, op1=OP.add,
                    )
                    stdr = wrk.tile([1, 512], f32, tag=f"stdr{h}", bufs=1,
                                    name=f"stdr{h}_{B}")
                    nc.scalar.activation(
                        stdr[:], v256r[:], AF.Sqrt, bias=epsq_sb[:],
                        scale=1.0 / 256.0,
                    )
                    rqrow = wrk.tile([1, 512], f32, tag=f"rqrow{h}", bufs=1,
                                     name=f"rqrow{h}_{B}")
                    nc.vector.reciprocal(rqrow[:], stdr[:])
                    mub[h] = wrk.tile([128, 512], f32, tag=f"mub{h}", bufs=1,
                                      name=f"mub{h}_{B}")
                    nc.gpsimd.partition_broadcast(mub[h][:], murow[:])
                    rqb[h] = wrk.tile([128, 512], f32, tag=f"rqb{h}", bufs=1,
                                      name=f"rqb{h}_{B}")
                    nc.gpsimd.partition_broadcast(rqb[h][:], rqrow[:])
                # --- normalize Q: ((raw + bias) - mu) * (1/std) -> bf16 ---
                for fc in range(4):
                    h = fc // 2
                    qtmp = wrk.tile([128, 512], f32, tag=f"qtmp{fc % 2}",
                                    name=f"qtmp{fc}_{B}")
                    nc.vector.scalar_tensor_tensor(
                        qtmp[:], qps_l[fc][:], bqc_sb[:, fc : fc + 1], mub[h][:],
                        op0=OP.add, op1=OP.subtract,
                    )
                    nc.gpsimd.tensor_mul(qtn[fc][B][:], qtmp[:], rqb[h][:])

        # ======== attention phase ========
        psA = ctx.enter_context(tc.tile_pool(name="psA", bufs=2, space="PSUM"))
        psB = ctx.enter_context(tc.tile_pool(name="psB", bufs=1, space="PSUM"))
        psT = ctx.enter_context(tc.tile_pool(name="psT", bufs=2, space="PSUM"))

        def attn_superblock(h, s, inject=None):
            n_k = 4 * (s + 1)
            oacc = [
                psB.tile([128, 258], f32, tag=f"oacc{j}", name=f"oacc{h}_{s}_{j}")
                for j in range(4)
            ]
            p_tiles = [None] * n_k

            def issue_scores(kt):
                d = kt - 4 * s  # >= 0 on the diagonal region
                qoff = 0 if d <= 0 else d * 128  # bf16 moving: 1 cyc/row anyway
                st = psA.tile([128, 512], f32, tag="mm512", name=f"st{h}_{s}_{kt}")
                for c in range(2):
                    nc.tensor.matmul(
                        st[:, qoff:512],
                        lhsT=ktb[c][kt // 4][:, (kt % 4) * 128 : (kt % 4 + 1) * 128],
                        rhs=qtn[h * 2 + c][s][:, qoff:512],
                        start=(c == 0),
                        stop=(c == 1),
                    )
                p = ppool.tile([128, 512], bf16, tag="p", name=f"p{h}_{s}_{kt}")
                nc.scalar.activation(
                    p[:, qoff:512], st[:, qoff:512], AF.Exp,
                    scale=rk16b[kt // 4][:, 2 * (kt % 4) : 2 * (kt % 4) + 1],
                )
                if d >= 0:
                    nc.vector.tensor_mul(
                        p[:, d * 128 : (d + 1) * 128],
                        p[:, d * 128 : (d + 1) * 128],
                        tmask_sb[:],
                    )
                p_tiles[kt] = p

            def issue_pv(kt):
                d = kt - 4 * s
                p = p_tiles[kt]
                for j in range(max(d, 0), 4):
                    nc.tensor.matmul(
                        oacc[j][:],
                        lhsT=p[:, j * 128 : (j + 1) * 128],
                        rhs=vp_t[kt][:],
                        start=(kt == 0),
                        stop=(kt == 4 * s + j),
                    )

            for kk in range(min(3, n_k)):
                issue_scores(kk)
            if inject is not None:
                inject()
            osb_list = [None] * 4

            def finish_j(j):
                rc = wrk.tile([128, 1], f32, tag="rc", name=f"rc{h}_{s}_{j}")
                nc.vector.reciprocal(rc[:], oacc[j][:, 256:257])
                osb = opool.tile([128, 256], bf16, tag="osb", name=f"osb{h}_{s}_{j}")
                nc.vector.tensor_scalar_mul(osb[:], oacc[j][:, 0:256], rc[:])
                osb_list[j] = osb

            for kt in range(n_k):
                issue_pv(kt)
                if kt + 3 < n_k:
                    issue_scores(kt + 3)
                if kt >= 4 * s:
                    finish_j(kt - 4 * s)
            return osb_list

        def o_trans(h, s, osb_list):
            """transpose the 4 normalized output tiles of (h, s) into otb"""
            for c in range(2):
                big = psT.tile([128, 512], bf16, tag="tp", name=f"obig{h}_{s}_{c}")
                for j in range(4):
                    nc.tensor.matmul(
                        big[:, j * 128 : (j + 1) * 128],
                        lhsT=osb_list[j][:, c * 128 : (c + 1) * 128],
                        rhs=ident_sb[:],
                        is_transpose=True,
                    )
                nc.vector.tensor_copy(otb[h * 2 + c][s][:], big[:])

        def o_proj(t):
            s, j = t // 4, t % 4
            ops = psT.tile([128, 256], f32, tag="tp", name=f"ops{t}")
            for c in range(4):
                nc.tensor.matmul(
                    ops[:],
                    lhsT=otb[c][s][:, j * 128 : (j + 1) * 128],
                    rhs=wo_sb[:, c, :],
                    start=(c == 0),
                    stop=(c == 3),
                )
            outsb = opool.tile([128, 256], f32, tag="outsb", name=f"outsb{t}")
            nc.scalar.copy(outsb[:], ops[:])
            nc.sync.dma_start(out_d[t * 128 : (t + 1) * 128, :], outsb[:])

        # pipeline: h0(s) transposes + h1(s-1) transposes + o_proj(s-1) all
        # run inside later instruction streams so their DVE inputs are ready.
        osb_mem = {}
        for s in range(NSB):

            def inj_h0(ss=s):
                if ss > 0:
                    o_trans(1, ss - 1, osb_mem[(1, ss - 1)])

            def inj_h1(ss=s):
                if ss > 0:
                    for t in range(4 * (ss - 1), 4 * ss):
                        o_proj(t)
                o_trans(0, ss, osb_mem[(0, ss)])

            osb_mem[(0, s)] = attn_superblock(0, s, inj_h0)
            osb_mem[(1, s)] = attn_superblock(1, s, inj_h1)

        # drain tail, j-granular: transpose/copy/project/store per query tile
        s_last = NSB - 1
        osb_l = osb_mem[(1, s_last)]
        for j in range(4):
            jb = psT.tile([128, 256], bf16, tag="tp", name=f"jb{j}")
            for c in range(2):
                nc.tensor.matmul(
                    jb[:, c * 128 : (c + 1) * 128],
                    lhsT=osb_l[j][:, c * 128 : (c + 1) * 128],
                    rhs=ident_sb[:],
                    is_transpose=True,
                )
            for c in range(2):
                nc.vector.tensor_copy(
                    otb[2 + c][s_last][:, j * 128 : (j + 1) * 128],
                    jb[:, c * 128 : (c + 1) * 128],
                )
            o_proj(4 * s_last + j)

    nc.finalize()
    return nc


def _chunk2(a):
    """[256, F] -> [128, 2, F] (feature chunks on the free axis)."""
    f = a.shape[1]
    return np.ascontiguousarray(a.reshape(2, 128, f).transpose(1, 0, 2))


def _prep_core_inputs(c, x, WK_w, WK_b, WV_w, WV_b, WQ_w, WQ_b, WO_w):
    import ml_dtypes

    bf16 = ml_dtypes.bfloat16
    b, g, j = c // 4, (c // 2) % 2, c % 2
    f32 = np.float32

    xT = np.ascontiguousarray(x[:, b, :].T.astype(f32))  # [256, 2048]
    xt = _chunk2(xT).astype(bf16)

    wk_s = WK_w[:, g * 256 : (g + 1) * 256].astype(f32)  # [256 in, 256 out]
    bk_s = WK_b[g * 256 : (g + 1) * 256].astype(f32)
    wv_s = np.ascontiguousarray(WV_w[:, g * 256 : (g + 1) * 256].astype(f32))

    col = (g * SUB + 2 * j) * 256
    wq_s = WQ_w[:, col : col + 512].astype(f32)  # both heads [256 in, 512 out]
    bq_s = WQ_b[col : col + 512].astype(f32)

    row = (g * SUB + 2 * j) * 256
    wo_s = WO_w[row : row + 512, :].astype(f32)  # [512, 256]
    wo = np.ascontiguousarray(wo_s.reshape(4, 128, 256).transpose(1, 0, 2))

    pp, ff = np.meshgrid(np.arange(128), np.arange(128), indexing="ij")
    hdr = np.concatenate(
        [
            np.ascontiguousarray(_chunk2(wk_s).reshape(128, 512)).astype(bf16),
            xt[:, 0, 0:512],
            xt[:, 1, 0:512],
            np.ascontiguousarray(_chunk2(wq_s).reshape(128, 1024)).astype(bf16),
        ],
        axis=1,
    )
    return {
        "xt": xt,
        "hdr": np.ascontiguousarray(hdr),
        "bkc": np.ascontiguousarray(bk_s.reshape(2, 128).T),
        "wmk": np.ascontiguousarray(
            np.repeat(wk_s.mean(axis=1, keepdims=True).reshape(2, 128).T, 2, axis=1)
        ).astype(bf16),
        "bkm": np.full((128, 1), bk_s.mean(), dtype=f32),
        "bqc": np.ascontiguousarray(bq_s.reshape(4, 128).T),
        "wmq": np.ascontiguousarray(
            wq_s.reshape(256, 2, 256).mean(axis=2).reshape(2, 128, 2).transpose(1, 0, 2)
        ).astype(bf16),
        "bqm": np.array(
            [[bq_s[0:256].mean(), bq_s[256:512].mean()]], dtype=f32
        ),
        "wv": _chunk2(wv_s).astype(bf16),
        "wo": wo.astype(bf16),
        "ident": np.eye(128, dtype=bf16),
        "cmask": (pp <= ff).astype(bf16),  # keep k<=q on the diagonal tile
        "vpones": np.concatenate(
            [np.ones((128, NT, 1), dtype=bf16), np.zeros((128, NT, 1), dtype=bf16)],
            axis=2,
        ),
    }


def kernel(x, WK_w, WK_b, WV_w, WV_b, WQ_w, WQ_b, WO_w, WO_b, ln_g, ln_b, **kwargs):
    x = np.asarray(x)
    WK_w, WK_b = np.asarray(WK_w), np.asarray(WK_b)
    WV_w, WV_b = np.asarray(WV_w), np.asarray(WV_b)
    WQ_w, WQ_b = np.asarray(WQ_w), np.asarray(WQ_b)
    WO_w, WO_b = np.asarray(WO_w), np.asarray(WO_b)
    ln_g, ln_b = np.asarray(ln_g), np.asarray(ln_b)

    if not np.allclose(ln_b, 0.0):
        raise NotImplementedError("nonzero ln_b not supported by this kernel")
    if not np.allclose(ln_g, 1.0):
        raise NotImplementedError("non-unit ln_g not supported by this kernel")

    if "nc" not in _CACHE:
        _CACHE["nc"] = _build_program()
    nc = _CACHE["nc"]

    in_maps = [
        _prep_core_inputs(c, x, WK_w, WK_b, WV_w, WV_b, WQ_w, WQ_b, WO_w)
        for c in range(N_CORES)
    ]

    from concourse.bass_utils import run_bass_kernel_spmd

    res = run_bass_kernel_spmd(nc, in_maps, list(range(N_CORES)))
    results = res.results

    out = np.zeros((SEQ, BS, DIM), dtype=np.float32)
    for c in range(N_CORES):
        out[:, c // 4, :] += results[c]["out_partial"]

    # fold: WO_b plus the V-bias contribution of every head
    const_bias = WO_b.astype(np.float64).copy()
    for g in range(G):
        bv = WV_b[g * 256 : (g + 1) * 256].astype(np.float64)
        for sh in range(SUB):
            row = (g * SUB + sh) * 256
            const_bias += bv @ WO_w[row : row + 256, :].astype(np.float64)
    out += const_bias.astype(np.float32)[None, None, :]
    return out


# revision 57
# speedup vs baseline: 1.5938x; 1.0125x over previous
"""GroupedAttention Trainium2 kernel.

Problem: x[2048, 2, 256]; K/V projections to G=2 groups (head width 256),
Q projection to G*SUB=8 heads; LayerNorm on K and Q; causal softmax
attention per (b, g, sub); output projection back to 256.

Sharding: 16 (b, g, sub) heads over 8 cores -> 2 heads per core.
Core c: b = c//4, g = (c//2)%2, sub-pair j = c%2 (subs 2j, 2j+1).
The host sums the 4 partials per batch and adds a folded constant bias.

Key structure (all timings against the TRN2 cost model):
- K^T and Q^T are produced DIRECTLY by matmuls (weights stationary,
  x^T moving), eliminating every K/Q transpose on the PE.
- LayerNorm is never applied to K. Scores use raw (biased) K^T; the
  per-key factor 1/(16*std_k) folds into the Exp activation's
  per-partition scale AP, and the mean term cancels because the
  normalized Q rows sum to ~0 (ln_g == 1). Per-key mean/sumsq come from
  tiny N=1 matmuls against precomputed row-mean weight columns and a
  Square+ones-reduction, batched 4 seq-tiles per PSUM bank.
- Q IS normalized (its per-query scale sits inside the softmax):
  mean/sumsq rows are computed by M=2/M=1 matmuls into one [4,512]
  PSUM tile, converted to (mu, 1/std) rows, broadcast across partitions
  on the otherwise-idle GPSIMD engine, and applied with one
  scalar_tensor_tensor (bias add + mean subtract) plus one multiply.
- Causal structure at 128-tile granularity: score columns below the
  diagonal tile are skipped entirely (bf16 moving keeps 1 cyc/row even
  below 256 columns), PV matmuls for empty tile pairs are skipped, and
  only the diagonal 128x128 tile is masked (one shared 0/1 bf16 mask).
- A ones-column appended to V makes PSUM column 256 accumulate the
  softmax denominator for free.
- The kt loop is software-pipelined (scores two blocks ahead of PV);
  O-transposes are batched 4-per-bank with one wide PSUM->SBUF copy and
  injected, with the output projection, into the next superblock's
  instruction stream so the PE never waits on DVE chains.
- bf16 for Q^T-normalized, P, O tiles and the transpose identity
  (transposes run 1.0 cyc/row); everything accumulates in fp32 PSUM.
"""

import sys

import numpy as np

for _p in ("/opt/trn_rl_repo",):
    if _p not in sys.path:
        sys.path.insert(0, _p)

SEQ, BS, DIM = 2048, 2, 256
G, SUB = 2, 4
N_CORES = 8
LN_EPS = 1e-5
NT = SEQ // 128  # 16 seq tiles of 128
NSB = SEQ // 512  # 4 blocks of 512 (query superblocks / proj blocks)

_CACHE = {}


def _build_program():
    from contextlib import ExitStack

    import concourse.bacc as bacc
    import concourse.bass_isa as bass_isa
    import concourse.mybir as mybir
    from concourse import tile
    f32 = mybir.dt.float32
    f32r = mybir.dt.float32r
    bf16 = mybir.dt.bfloat16
    AF = mybir.ActivationFunctionType
    OP = mybir.AluOpType

    nc = bacc.Bacc("TRN2", target_bir_lowering=False, debug=False)

    xt_d = nc.dram_tensor("xt", [128, 2, SEQ], bf16, kind="ExternalInput").ap()
    hdr_d = nc.dram_tensor("hdr", [128, 2560], bf16, kind="ExternalInput").ap()
    bkc_d = nc.dram_tensor("bkc", [128, 2], f32, kind="ExternalInput").ap()
    wmk_d = nc.dram_tensor("wmk", [128, 4], bf16, kind="ExternalInput").ap()
    bkm_d = nc.dram_tensor("bkm", [128, 1], f32, kind="ExternalInput").ap()
    bqc_d = nc.dram_tensor("bqc", [128, 4], f32, kind="ExternalInput").ap()
    wmq_d = nc.dram_tensor("wmq", [128, 2, 2], bf16, kind="ExternalInput").ap()
    bqm_d = nc.dram_tensor("bqm", [1, 2], f32, kind="ExternalInput").ap()
    wv_d = nc.dram_tensor("wv", [128, 2, 256], bf16, kind="ExternalInput").ap()
    wo_d = nc.dram_tensor("wo", [128, 4, 256], bf16, kind="ExternalInput").ap()
    id_d = nc.dram_tensor("ident", [128, 128], bf16, kind="ExternalInput").ap()
    cm_d = nc.dram_tensor("cmask", [128, 128], bf16, kind="ExternalInput").ap()
    vo_d = nc.dram_tensor("vpones", [128, NT, 2], bf16, kind="ExternalInput").ap()
    out_d = nc.dram_tensor("out_partial", [SEQ, DIM], f32, kind="ExternalOutput").ap()

    r = lambda ap: ap.bitcast(f32r)

    with tile.TileContext(nc) as tc, ExitStack() as ctx:
        const = ctx.enter_context(tc.tile_pool(name="const", bufs=1))

        xt_sb = const.tile([128, 2, SEQ], bf16)
        hdr_sb = const.tile([128, 2560], bf16)
        bkc_sb = const.tile([128, 2], f32)
        wmk_sb = const.tile([128, 4], bf16)
        bkm_sb = const.tile([128, 1], f32)
        bqc_sb = const.tile([128, 4], f32)
        wmq_sb = const.tile([128, 2, 2], bf16)
        bqm_sb = const.tile([1, 2], f32)
        wv_sb = const.tile([128, 2, 256], bf16)
        wo_sb = const.tile([128, 4, 256], bf16)
        ident_sb = const.tile([128, 128], bf16)
        tmask_sb = const.tile([128, 128], bf16)
        onescol_sb = const.tile([128, 2], f32)
        epsk_sb = const.tile([128, 1], f32)
        epsq_sb = const.tile([1, 1], f32)

        # persistent SBUF activations
        ktb = [
            [const.tile([128, 512], bf16, name=f"ktb{f}_{b}") for b in range(NSB)]
            for f in range(2)
        ]
        qtn = [
            [const.tile([128, 512], bf16, name=f"qtn{fc}_{b}") for b in range(NSB)]
            for fc in range(4)
        ]
        vp_t = [const.tile([128, 258], bf16, name=f"vpt{t}") for t in range(NT)]
        rk16b = [const.tile([128, 8], f32, name=f"rk16b{b}") for b in range(NSB)]
        otb = [
            [const.tile([128, 512], bf16, name=f"otb{c}_{s}") for s in range(NSB)]
            for c in range(4)
        ]

        nc.gpsimd.memset(onescol_sb[:], 1.0)
        nc.gpsimd.memset(epsk_sb[:], 256.0 * LN_EPS)
        nc.gpsimd.memset(epsq_sb[:], LN_EPS)
        # prime the sqrt-capable activation table before any Square lands
        warm_sb = const.tile([1, 1], f32)
        nc.scalar.activation(warm_sb[:], epsq_sb[:], AF.Sqrt)
        nc.sync.dma_start(hdr_sb[:, 0:512], hdr_d[:, 0:512])
        nc.sync.dma_start(hdr_sb[:, 512:1024], hdr_d[:, 512:1024])
        nc.sync.dma_start(hdr_sb[:, 1024:1536], hdr_d[:, 1024:1536])
        nc.sync.dma_start(hdr_sb[:, 1536:2560], hdr_d[:, 1536:2560])
        nc.sync.dma_start(bkc_sb[:], bkc_d[:])
        nc.sync.dma_start(wmk_sb[:], wmk_d[:])
        nc.sync.dma_start(bkm_sb[:], bkm_d[:])
        nc.sync.dma_start(bqc_sb[:], bqc_d[:])
        nc.sync.dma_start(wmq_sb[:], wmq_d[:])
        nc.sync.dma_start(bqm_sb[:], bqm_d[:])
        nc.sync.dma_start(wv_sb[:], wv_d[:])
        nc.sync.dma_start(xt_sb[:, :, 512:1024], xt_d[:, :, 512:1024])
        nc.sync.dma_start(xt_sb[:, :, 1024:1536], xt_d[:, :, 1024:1536])
        nc.sync.dma_start(xt_sb[:, :, 1536:2048], xt_d[:, :, 1536:2048])
        nc.sync.dma_start(ident_sb[:], id_d[:])
        nc.sync.dma_start(tmask_sb[:], cm_d[:])
        for t in range(NT):
            nc.sync.dma_start(vp_t[t][:, 256:258], vo_d[:, t, :])
        nc.sync.dma_start(wo_sb[:], wo_d[:])


        def wk_v(c, f):
            return hdr_sb[:, c * 256 + f * 128 : c * 256 + (f + 1) * 128]

        def wq_v(c, lo, hi):
            return hdr_sb[:, 1536 + c * 512 + lo : 1536 + c * 512 + hi]

        def xt_v(c, lo, hi):
            if hi <= 512:
                base = 512 + c * 512
                return hdr_sb[:, base + lo : base + hi]
            return xt_sb[:, c, lo:hi]

        wrk = ctx.enter_context(tc.tile_pool(name="wrk", bufs=3))
        ppool = ctx.enter_context(tc.tile_pool(name="ppool", bufs=6))
        opool = ctx.enter_context(tc.tile_pool(name="opool", bufs=6))

        # ======== projection phase (pools scoped; close before attention) ====
        with tc.tile_pool(name="projp", bufs=3, space="PSUM") as projp, \
             tc.tile_pool(name="tinyp", bufs=1, space="PSUM") as tinyp, \
             tc.tile_pool(name="rowp", bufs=1, space="PSUM") as rowp:
            for B in range(NSB):
                sl = slice(B * 512, (B + 1) * 512)
                # --- K^T chunks: biased SBUF copy + biased square (from PSUM)
                ktsq = []
                for f in range(2):
                    kps = projp.tile([128, 512], f32, tag="proj", name=f"kps{f}_{B}")
                    for c in range(2):
                        nc.tensor.matmul(
                            kps[:],
                            lhsT=wk_v(c, f),
                            rhs=xt_v(c, B * 512, (B + 1) * 512),
                            start=(c == 0),
                            stop=(c == 1),
                        )
                    ksq = wrk.tile([128, 512], f32r, tag=f"ksq{f}", bufs=2,
                                   name=f"ksq{f}_{B}")
                    nc.scalar.activation(
                        ksq[:], kps[:], AF.Square, bias=bkc_sb[:, f : f + 1]
                    )
                    nc.vector.tensor_scalar_add(
                        ktb[f][B][:], kps[:], scalar1=bkc_sb[:, f : f + 1]
                    )
                    ktsq.append(ksq)
                # --- Q^T chunks: raw in PSUM until normalize; biased square
                qps_l, qtsq = [], []
                for fc in range(4):
                    qps = projp.tile([128, 512], f32, tag="proj", name=f"qps{fc}_{B}")
                    for c in range(2):
                        nc.tensor.matmul(
                            qps[:],
                            lhsT=wq_v(c, fc * 128, (fc + 1) * 128),
                            rhs=xt_v(c, B * 512, (B + 1) * 512),
                            start=(c == 0),
                            stop=(c == 1),
                        )
                    qsq = wrk.tile([128, 512], f32r, tag=f"qsq{fc}", bufs=2,
                                   name=f"qsq{fc}_{B}")
                    nc.scalar.activation(
                        qsq[:], qps[:], AF.Square, bias=bqc_sb[:, fc : fc + 1]
                    )
                    qps_l.append(qps)
                    qtsq.append(qsq)
                # --- V tiles ---
                for t in range(4 * B, 4 * B + 4):
                    vps = projp.tile([128, 256], f32, tag="proj", name=f"vps{t}")
                    for c in range(2):
                        nc.tensor.matmul(
                            vps[:],
                            lhsT=xt_v(c, t * 128, (t + 1) * 128),
                            rhs=wv_sb[:, c, :],
                            start=(c == 0),
                            stop=(c == 1),
                        )
                    nc.scalar.copy(vp_t[t][:, 0:256], vps[:])
                # --- K per-key stats: mu (cols 0:4) and sumsq (cols 4:8) ---
                tiny = tinyp.tile([128, 16], f32, tag="tiny", name=f"tiny{B}")
                for i, t in enumerate(range(4 * B, 4 * B + 4)):
                    for c in range(2):
                        nc.tensor.matmul(
                            tiny[:, 2 * i : 2 * i + 2],
                            lhsT=xt_v(c, t * 128, (t + 1) * 128),
                            rhs=wmk_sb[:, 2 * c : 2 * c + 2],
                            start=(c == 0),
                            stop=(c == 1),
                        )
                    for f in range(2):
                        nc.tensor.matmul(
                            tiny[:, 8 + 2 * i : 10 + 2 * i],
                            lhsT=r(ktsq[f][:, i * 128 : (i + 1) * 128]),
                            rhs=r(onescol_sb[:, 0:2]),
                            start=(f == 0),
                            stop=(f == 1),
                        )
                # --- Q row stats, one bank per head: mu at partition 0,
                # sumsq at partition 32 (matmul base must be 0/32/64)
                mur_h = [
                    rowp.tile([1, 512], f32, tag=f"mur{h}", name=f"mur{h}_{B}")[:]
                    for h in range(2)
                ]
                for h in range(2):
                    for c in range(2):
                        nc.tensor.matmul(
                            mur_h[h],
                            lhsT=wmq_sb[:, c, h : h + 1],
                            rhs=xt_v(c, B * 512, (B + 1) * 512),
                            start=(c == 0),
                            stop=(c == 1),
                        )
                sqr_h = [
                    rowp.tile([1, 512], f32, tag=f"sqr{h}", name=f"sqr{h}_{B}")[:]
                    for h in range(2)
                ]
                for h in range(2):
                    for c in range(2):
                        nc.tensor.matmul(
                            sqr_h[h],
                            lhsT=r(onescol_sb[:, 0:1]),
                            rhs=r(qtsq[2 * h + c][:]),
                            start=(c == 0),
                            stop=(c == 1),
                        )
                # --- K stats -> rk16 (batched over the 4 seq tiles) ---
                mu2 = wrk.tile([128, 8], f32, tag="mu2", name=f"mu2_{B}")
                nc.scalar.activation(mu2[:], tiny[:, 0:8], AF.Square, bias=bkm_sb[:])
                v256 = wrk.tile([128, 8], f32, tag="v256", name=f"v256_{B}")
                nc.vector.scalar_tensor_tensor(
                    v256[:], mu2[:], -256.0, tiny[:, 8:16],
                    op0=OP.mult, op1=OP.add,
                )
                std16 = wrk.tile([128, 8], f32, tag="std16", name=f"std16_{B}")
                nc.scalar.activation(std16[:], v256[:], AF.Sqrt, bias=epsk_sb[:])
                nc.vector.reciprocal(rk16b[B][:], std16[:])
                # --- Q row stats -> (mu_biased, 1/std) rows + broadcasts ---
                mub = [None, None]
                rqb = [None, None]
                for h in range(2):
                    murow = wrk.tile([1, 512], f32, tag=f"murow{h}", bufs=1,
                                     name=f"murow{h}_{B}")
                    nc.vector.tensor_scalar_add(
                        murow[:], mur_h[h],
                        scalar1=bqm_sb[0:1, h : h + 1],
                    )
                    mu2r = wrk.tile([1, 512], f32, tag=f"mu2r{h}", bufs=1,
                                    name=f"mu2r{h}_{B}")
                    nc.scalar.activation(mu2r[:], murow[:], AF.Square)
                    v256r = wrk.tile([1, 512], f32, tag=f"v256r{h}", bufs=1,
                                     name=f"v256r{h}_{B}")
                    nc.vector.scalar_tensor_tensor(
                        v256r[:], mu2r[:], -256.0, sqr_h[h],
                        op0=OP.mult, op1=OP.add,
                    )
                    stdr = wrk.tile([1, 512], f32, tag=f"stdr{h}", bufs=1,
                                    name=f"stdr{h}_{B}")
                    nc.scalar.activation(
                        stdr[:], v256r[:], AF.Sqrt, bias=epsq_sb[:],
                        scale=1.0 / 256.0,
                    )
                    rqrow = wrk.tile([1, 512], f32, tag=f"rqrow{h}", bufs=1,
                                     name=f"rqrow{h}_{B}")
                    nc.vector.reciprocal(rqrow[:], stdr[:])
                    mub[h] = wrk.tile([128, 512], f32, tag=f"mub{h}", bufs=1,
                                      name=f"mub{h}_{B}")
                    nc.gpsimd.partition_broadcast(mub[h][:], murow[:])
                    rqb[h] = wrk.tile([128, 512], f32, tag=f"rqb{h}", bufs=1,
                                      name=f"rqb{h}_{B}")
                    nc.gpsimd.partition_broadcast(rqb[h][:], rqrow[:])
                # --- normalize Q: ((raw + bias) - mu) * (1/std) -> bf16 ---
                for fc in range(4):
                    h = fc // 2
                    qtmp = wrk.tile([128, 512], f32, tag=f"qtmp{fc % 2}",
                                    name=f"qtmp{fc}_{B}")
                    nc.vector.scalar_tensor_tensor(
                        qtmp[:], qps_l[fc][:], bqc_sb[:, fc : fc + 1], mub[h][:],
                        op0=OP.add, op1=OP.subtract,
                    )
                    nc.gpsimd.tensor_mul(qtn[fc][B][:], qtmp[:], rqb[h][:])

        # ======== attention phase ========
        psA = ctx.enter_context(tc.tile_pool(name="psA", bufs=2, space="PSUM"))
        psB = ctx.enter_context(tc.tile_pool(name="psB", bufs=1, space="PSUM"))
        psT = ctx.enter_context(tc.tile_pool(name="psT", bufs=2, space="PSUM"))

        def attn_superblock(h, s, inject=None):
            n_k = 4 * (s + 1)
            oacc = [
                psB.tile([128, 258], f32, tag=f"oacc{j}", name=f"oacc{h}_{s}_{j}")
                for j in range(4)
            ]
            p_tiles = [None] * n_k

            def issue_scores(kt):
                d = kt - 4 * s  # >= 0 on the diagonal region
                qoff = 0 if d <= 0 else d * 128  # bf16 moving: 1 cyc/row anyway
                st = psA.tile([128, 512], f32, tag="mm512", name=f"st{h}_{s}_{kt}")
                for c in range(2):
                    nc.tensor.matmul(
                        st[:, qoff:512],
                        lhsT=ktb[c][kt // 4][:, (kt % 4) * 128 : (kt % 4 + 1) * 128],
                        rhs=qtn[h * 2 + c][s][:, qoff:512],
                        start=(c == 0),
                        stop=(c == 1),
                    )
                p = ppool.tile([128, 512], bf16, tag="p", name=f"p{h}_{s}_{kt}")
                nc.scalar.activation(
                    p[:, qoff:512], st[:, qoff:512], AF.Exp,
                    scale=rk16b[kt // 4][:, 2 * (kt % 4) : 2 * (kt % 4) + 1],
                )
                if d >= 0:
                    nc.vector.tensor_mul(
                        p[:, d * 128 : (d + 1) * 128],
                        p[:, d * 128 : (d + 1) * 128],
                        tmask_sb[:],
                    )
                p_tiles[kt] = p

            def issue_pv(kt):
                d = kt - 4 * s
                p = p_tiles[kt]
                for j in range(max(d, 0), 4):
                    nc.tensor.matmul(
                        oacc[j][:],
                        lhsT=p[:, j * 128 : (j + 1) * 128],
                        rhs=vp_t[kt][:],
                        start=(kt == 0),
                        stop=(kt == 4 * s + j),
                    )

            for kk in range(min(3, n_k)):
                issue_scores(kk)
            if inject is not None:
                inject()
            osb_list = [None] * 4

            def finish_j(j):
                rc = wrk.tile([128, 1], f32, tag="rc", name=f"rc{h}_{s}_{j}")
                nc.vector.reciprocal(rc[:], oacc[j][:, 256:257])
                osb = opool.tile([128, 256], bf16, tag="osb", name=f"osb{h}_{s}_{j}")
                nc.vector.tensor_scalar_mul(osb[:], oacc[j][:, 0:256], rc[:])
                osb_list[j] = osb

            for kt in range(n_k):
                issue_pv(kt)
                if kt + 3 < n_k:
                    issue_scores(kt + 3)
                if kt >= 4 * s:
                    finish_j(kt - 4 * s)
            return osb_list

        def o_trans(h, s, osb_list):
            """transpose the 4 normalized output tiles of (h, s) into otb"""
            for c in range(2):
                big = psT.tile([128, 512], bf16, tag="tp", name=f"obig{h}_{s}_{c}")
                for j in range(4):
                    nc.tensor.matmul(
                        big[:, j * 128 : (j + 1) * 128],
                        lhsT=osb_list[j][:, c * 128 : (c + 1) * 128],
                        rhs=ident_sb[:],
                        is_transpose=True,
                    )
                nc.vector.tensor_copy(otb[h * 2 + c][s][:], big[:])

        def o_proj(t):
            s, j = t // 4, t % 4
            ops = psT.tile([128, 256], f32, tag="tp", name=f"ops{t}")
            for c in range(4):
                nc.tensor.matmul(
                    ops[:],
                    lhsT=otb[c][s][:, j * 128 : (j + 1) * 128],
                    rhs=wo_sb[:, c, :],
                    start=(c == 0),
                    stop=(c == 3),
                )
            outsb = opool.tile([128, 256], f32, tag="outsb", name=f"outsb{t}")
            nc.scalar.copy(outsb[:], ops[:])
            nc.sync.dma_start(out_d[t * 128 : (t + 1) * 128, :], outsb[:])

        # pipeline: h0(s) transposes + h1(s-1) transposes + o_proj(s-1) all
        # run inside later instruction streams so their DVE inputs are ready.
        osb_mem = {}
        for s in range(NSB):

            def inj_h0(ss=s):
                if ss > 0:
                    o_trans(1, ss - 1, osb_mem[(1, ss - 1)])

            def inj_h1(ss=s):
                if ss > 0:
                    for t in range(4 * (ss - 1), 4 * ss):
                        o_proj(t)
                o_trans(0, ss, osb_mem[(0, ss)])

            osb_mem[(0, s)] = attn_superblock(0, s, inj_h0)
            osb_mem[(1, s)] = attn_superblock(1, s, inj_h1)

        # drain tail, j-granular: transpose/copy/project/store per query tile
        s_last = NSB - 1
        osb_l = osb_mem[(1, s_last)]

        def tail_trans(j):
            jb = psT.tile([128, 256], bf16, tag="tp", name=f"jb{j}")
            for c in range(2):
                nc.tensor.matmul(
                    jb[:, c * 128 : (c + 1) * 128],
                    lhsT=osb_l[j][:, c * 128 : (c + 1) * 128],
                    rhs=ident_sb[:],
                    is_transpose=True,
                )
            for c in range(2):
                nc.vector.tensor_copy(
                    otb[2 + c][s_last][:, j * 128 : (j + 1) * 128],
                    jb[:, c * 128 : (c + 1) * 128],
                )

        tail_trans(0)
        tail_trans(1)
        for j in range(4):
            if j + 2 < 4:
                tail_trans(j + 2)
            o_proj(4 * s_last + j)

    nc.finalize()
    return nc


def _chunk2(a):
    """[256, F] -> [128, 2, F] (feature chunks on the free axis)."""
    f = a.shape[1]
    return np.ascontiguousarray(a.reshape(2, 128, f).transpose(1, 0, 2))


def _prep_core_inputs(c, x, WK_w, WK_b, WV_w, WV_b, WQ_w, WQ_b, WO_w):
    import ml_dtypes

    bf16 = ml_dtypes.bfloat16
    b, g, j = c // 4, (c // 2) % 2, c % 2
    f32 = np.float32

    xT = np.ascontiguousarray(x[:, b, :].T.astype(f32))  # [256, 2048]
    xt = _chunk2(xT).astype(bf16)

    wk_s = WK_w[:, g * 256 : (g + 1) * 256].astype(f32)  # [256 in, 256 out]
    bk_s = WK_b[g * 256 : (g + 1) * 256].astype(f32)
    wv_s = np.ascontiguousarray(WV_w[:, g * 256 : (g + 1) * 256].astype(f32))

    col = (g * SUB + 2 * j) * 256
    wq_s = WQ_w[:, col : col + 512].astype(f32)  # both heads [256 in, 512 out]
    bq_s = WQ_b[col : col + 512].astype(f32)

    row = (g * SUB + 2 * j) * 256
    wo_s = WO_w[row : row + 512, :].astype(f32)  # [512, 256]
    wo = np.ascontiguousarray(wo_s.reshape(4, 128, 256).transpose(1, 0, 2))

    pp, ff = np.meshgrid(np.arange(128), np.arange(128), indexing="ij")
    hdr = np.concatenate(
        [
            np.ascontiguousarray(_chunk2(wk_s).reshape(128, 512)).astype(bf16),
            xt[:, 0, 0:512],
            xt[:, 1, 0:512],
            np.ascontiguousarray(_chunk2(wq_s).reshape(128, 1024)).astype(bf16),
        ],
        axis=1,
    )
    return {
        "xt": xt,
        "hdr": np.ascontiguousarray(hdr),
        "bkc": np.ascontiguousarray(bk_s.reshape(2, 128).T),
        "wmk": np.ascontiguousarray(
            np.repeat(wk_s.mean(axis=1, keepdims=True).reshape(2, 128).T, 2, axis=1)
        ).astype(bf16),
        "bkm": np.full((128, 1), bk_s.mean(), dtype=f32),
        "bqc": np.ascontiguousarray(bq_s.reshape(4, 128).T),
        "wmq": np.ascontiguousarray(
            wq_s.reshape(256, 2, 256).mean(axis=2).reshape(2, 128, 2).transpose(1, 0, 2)
        ).astype(bf16),
        "bqm": np.array(
            [[bq_s[0:256].mean(), bq_s[256:512].mean()]], dtype=f32
        ),
        "wv": _chunk2(wv_s).astype(bf16),
        "wo": wo.astype(bf16),
        "ident": np.eye(128, dtype=bf16),
        "cmask": (pp <= ff).astype(bf16),  # keep k<=q on the diagonal tile
        "vpones": np.concatenate(
            [np.ones((128, NT, 1), dtype=bf16), np.zeros((128, NT, 1), dtype=bf16)],
            axis=2,
        ),
    }


def kernel(x, WK_w, WK_b, WV_w, WV_b, WQ_w, WQ_b, WO_w, WO_b, ln_g, ln_b, **kwargs):
    x = np.asarray(x)
    WK_w, WK_b = np.asarray(WK_w), np.asarray(WK_b)
    WV_w, WV_b = np.asarray(WV_w), np.asarray(WV_b)
    WQ_w, WQ_b = np.asarray(WQ_w), np.asarray(WQ_b)
    WO_w, WO_b = np.asarray(WO_w), np.asarray(WO_b)
    ln_g, ln_b = np.asarray(ln_g), np.asarray(ln_b)

    if not np.allclose(ln_b, 0.0):
        raise NotImplementedError("nonzero ln_b not supported by this kernel")
    if not np.allclose(ln_g, 1.0):
        raise NotImplementedError("non-unit ln_g not supported by this kernel")

    if "nc" not in _CACHE:
        _CACHE["nc"] = _build_program()
    nc = _CACHE["nc"]

    in_maps = [
        _prep_core_inputs(c, x, WK_w, WK_b, WV_w, WV_b, WQ_w, WQ_b, WO_w)
        for c in range(N_CORES)
    ]

    from concourse.bass_utils import run_bass_kernel_spmd

    res = run_bass_kernel_spmd(nc, in_maps, list(range(N_CORES)))
    results = res.results

    out = np.zeros((SEQ, BS, DIM), dtype=np.float32)
    for c in range(N_CORES):
        out[:, c // 4, :] += results[c]["out_partial"]

    # fold: WO_b plus the V-bias contribution of every head
    const_bias = WO_b.astype(np.float64).copy()
    for g in range(G):
        bv = WV_b[g * 256 : (g + 1) * 256].astype(np.float64)
        for sh in range(SUB):
            row = (g * SUB + sh) * 256
            const_bias += bv @ WO_w[row : row + 256, :].astype(np.float64)
    out += const_bias.astype(np.float32)[None, None, :]
    return out
